# revision 16
# baseline (speedup 1.0000x reference)
"""Fused transformer block (LN -> causal MHA -> residual -> LN -> SiLU MLP -> residual)
on 8 Trainium2 NeuronCores.

v3 design (on top of the v2 baseline):
- Tensor-parallel over heads (2 heads/core) for QKV + attention; S computed
  transposed (S^T) with the softmax denominator as a ones-column of V.
- Attention outputs are TRANSPOSED and quantized to fp8 BEFORE the AllToAll
  (payload halves to 512KB/group; after the exchange the received buffer is
  directly the lhsT of the O-projection -> no post-collective PE work beyond
  the matmuls themselves).
- x2 (attention residual) is kept in SBUF in f32, scaled by 32 so that the
  fp8(32*W2) and bf16(32*W2) MLP2 products accumulate uniformly; the final
  drain rescales by 1/32. LN2 is scale-invariant (eps scaled to match).
- MLP is FUSED: u = silu(mlp1) stays in SBUF (aliased onto the dead K/Q
  SBUF region); no DRAM round trip. MLP2 runs in 4 feature passes of 4 PSUM
  banks each.
- Mixed-precision MLP: mid-tiles 0..19 of MLP1 and mid-tile pairs 0..10 of
  MLP2 run fp8+DoubleRow (1.8x); the rest bf16. Chosen so the predicted
  rel-err (numpy-emulated, matches HW to 3 digits) is ~1.8e-2 < 2e-2.
- The first 20 MLP1 tiles run as two 256-token halves so their first halves
  (token chunks 0,1, available right after C2(1)) fill the ~50us AllToAll #2
  latency window; their fp8 W1 tiles are streamed twice (5MB extra).
- PSUM ring decoupling: attention (ring qs/sm), C (own ring c), so the
  collective-dependent O-projection never blocks attention PSUM reuse.
- Output written bf16 (host upcasts), xres folded with b2 and pre-scaled on
  the host.
"""

import sys
import os

for _p in ("/opt/trn_rl_repo", "/root/.axon_site/_ro/trn_rl_repo"):
    if os.path.isdir(_p) and _p not in sys.path:
        sys.path.insert(0, _p)
        break

import numpy as np
import ml_dtypes

import concourse.bass as bass
from concourse import bacc
import concourse.mybir as mybir
import concourse.tile as tile
from concourse.masks import make_identity
from concourse.bass_utils import run_bass_kernel_spmd

F32 = mybir.dt.float32
BF16 = mybir.dt.bfloat16
FP8 = mybir.dt.float8e4


def _install_act_table_hint():
    """Steer the act-table-set chooser so Exp and Ln resolve to the one set
    that contains BOTH (natural_log_exp_and_others)."""
    import concourse.bacc as _bacc
    if getattr(_bacc, "_act_hint_installed", False):
        return
    _orig = _bacc.get_activation_tables

    def _patched(arch):
        tabs = _orig(arch)
        exp = mybir.ActivationFunctionType.Exp
        ln = mybir.ActivationFunctionType.Ln
        for name, fns in tabs.items():
            if name != "natural_log_exp_and_others":
                fns.discard(exp)
                fns.discard(ln)
        return tabs

    _bacc.get_activation_tables = _patched
    _bacc._act_hint_installed = True


_install_act_table_hint()

P = 128          # partitions / head_dim / token tile
H = 2048         # hidden
KS = H // P      # 16 k-subtiles over hidden
HEADS = 16
HL = 2           # heads per core
NCORES = 8
B = 2
T = 2048
NTOK = B * T     # 4096
TPB = T          # tokens per batch
MID = 4 * H      # 8192
MMT = MID // P   # 64 m-tiles over mid dim
DQK = 2 * HL * P   # 512 rows of fused QK projection per core
DV = HL * P        # 256 V/attention-out features per core
EPS = 1e-5
NEG = -1.0e30

QT_PER_B = TPB // P   # 16 q tiles per batch
MT = NTOK // P        # 32 token m-tiles
NCHUNK = 4            # token chunks per core (128 each)
GT = 256              # tokens per A-group
WSCALE = 16.0         # fp8 weight rescale (avoids e4m3 subnormals)
SCORE_SCALE = (1.0 / np.sqrt(P)) / (WSCALE * WSCALE)
NG_PER_B = TPB // GT  # 8 A-groups per batch

# ---- mixed-precision MLP config ----
N1F = 20              # MLP1 mid-tiles 0..N1F-1 in fp8 DoubleRow (also the
                      # "early" tiles run as two 256-token halves)
N2P = 11              # MLP2 mid-tile PAIRS 0..N2P-1 (tiles 0..21) in fp8 DR
N2F = 2 * N2P         # fp8 MLP2 tiles
NBF2 = MMT - N2F      # 42 bf16 MLP2 tiles
NPASS = 4             # MLP2 feature passes (512 cols each)
X2S = 16.0            # x2 kept as 16*x2_true in SBUF


def build(sim=False, trn_kwargs=None, trace_sim=False):
    nc = bacc.Bacc(None, num_devices=NCORES, **(trn_kwargs or {}))

    x_d = nc.declare_dram_parameter("xbf", [NTOK, H], BF16, isOutput=False)
    xres_d = nc.declare_dram_parameter("xres", [NCHUNK * P, H], F32, isOutput=False)
    wqk_d = nc.declare_dram_parameter("wqk", [P, KS, DQK], FP8, isOutput=False)
    bqk_d = nc.declare_dram_parameter("bqk", [P, DQK // P], F32, isOutput=False)
    wv_d = nc.declare_dram_parameter("wv", [P, KS, DV], FP8, isOutput=False)
    bvbc_d = nc.declare_dram_parameter("bvbc", [P, HL, P], F32, isOutput=False)
    wo_d = nc.declare_dram_parameter("wo", [P, KS, H], FP8, isOutput=False)
    w1f8_d = nc.declare_dram_parameter("w1f8", [N1F, P, KS, P], FP8, isOutput=False)
    w1bf_d = nc.declare_dram_parameter("w1bf", [MMT - N1F, P, KS, P], BF16,
                                       isOutput=False)
    b1_d = nc.declare_dram_parameter("b1", [P, MMT], F32, isOutput=False)
    w2f8_d = nc.declare_dram_parameter("w2f8", [NPASS, N2P, P, 2, 512], FP8,
                                       isOutput=False)
    w2bf_d = nc.declare_dram_parameter("w2bf", [NPASS, NBF2, P, 512], BF16,
                                       isOutput=False)
    cmaskT_d = nc.declare_dram_parameter("cmaskT", [P, P], F32, isOutput=False)
    out_d = nc.declare_dram_parameter("out", [NCHUNK * P, H], BF16, isOutput=True)

    from contextlib import ExitStack
    with tile.TileContext(nc, trace_sim=trace_sim) as tc:
        with ExitStack() as stack:
            dram = stack.enter_context(tc.tile_pool(name="dram", bufs=1, space="DRAM"))
            const = stack.enter_context(tc.tile_pool(name="const", bufs=1))
            wbig = stack.enter_context(tc.tile_pool(name="wbig", bufs=1))
            # wqk (8KB/part, dead after QKV) chained with h2T (16KB)
            p_ali = stack.enter_context(tc.tile_pool(name="ali16", bufs=1))
            # ksb+qT (32KB, dead after last AV) chained with ubf (42KB)
            p_kvu = stack.enter_context(tc.tile_pool(name="kvu", bufs=1))
            p_vsb = stack.enter_context(tc.tile_pool(name="vsb", bufs=2))
            p_u8 = stack.enter_context(tc.tile_pool(name="u8", bufs=1))
            p_x = stack.enter_context(tc.tile_pool(name="xin", bufs=2))
            p_ln = stack.enter_context(tc.tile_pool(name="lnsmall", bufs=2))
            p_h = stack.enter_context(tc.tile_pool(name="htok", bufs=2))
            p_hT = stack.enter_context(tc.tile_pool(name="hT", bufs=2))
            p_h2T8 = stack.enter_context(tc.tile_pool(name="h2T8", bufs=1))
            p_ex = stack.enter_context(tc.tile_pool(name="expT", bufs=2))
            p_ao = stack.enter_context(tc.tile_pool(name="aot", bufs=2))
            p_aoT = stack.enter_context(tc.tile_pool(name="aoT", bufs=1))
            p_afT = stack.enter_context(tc.tile_pool(name="afT", bufs=1))
            p_x2 = stack.enter_context(tc.tile_pool(name="x2", bufs=4))
            p_w1 = stack.enter_context(tc.tile_pool(name="w1pool", bufs=2))
            p_w2 = stack.enter_context(tc.tile_pool(name="w2pool", bufs=2))
            # PSUM rings:
            #  qs: 4 x 2KB  (A-QK, A-transposes, B-S, D-full psU, E-psY)
            #  sm: 4 x 1KB  (A-V, B-AV, B-aoT transposes, D-early psU halves)
            #  c : 2 x 2KB  (C O-proj, C2 h2 transposes) -- collective-coupled
            ps_qs = stack.enter_context(tc.tile_pool(name="psqs", bufs=4, space="PSUM"))
            ps_sm = stack.enter_context(tc.tile_pool(name="pssm", bufs=2, space="PSUM"))
            ps_c = stack.enter_context(tc.tile_pool(name="psc", bufs=2, space="PSUM"))

            # ---- internal DRAM ----
            # aotT laid [g][dst s][jj][fsub][f][t]; per-(g,s) shard contiguous
            aot_dram = dram.tile([2, NCORES * 2 * 2 * P, P], BF16)
            a2a_dram = dram.tile([2, NCORES * 2 * 2 * P, P], BF16)

            # ---- constants / weights in SBUF ----
            ident = const.tile([P, P], BF16)
            make_identity(nc, ident)
            epsb = const.tile([P, 1], F32)
            nc.vector.memset(epsb[:], EPS)
            epsb2 = const.tile([P, 1], F32)
            nc.vector.memset(epsb2[:], EPS * X2S * X2S)
            scrap = const.tile([P, 1], F32)
            cmaskT = const.tile([P, P], F32)
            nc.sync.dma_start(cmaskT[:], cmaskT_d[:, :])
            bqk_sb = const.tile([P, DQK // P], F32)
            nc.sync.dma_start(bqk_sb[:], bqk_d[:, :])
            bvbc_sb = const.tile([P, HL, P], F32)
            nc.sync.dma_start(bvbc_sb[:], bvbc_d[:, :, :])
            b1_sb = const.tile([P, MMT], F32)
            nc.sync.dma_start(b1_sb[:], b1_d[:, :])
            wqk_sb = p_ali.tile([P, KS, DQK], FP8, tag="ali16", name="wqk_sb")
            wv_sb = wbig.tile([P, KS, DV], FP8)
            wo_sb = wbig.tile([P, KS, H], FP8)

            def emit_weight_dmas():
                nc.gpsimd.dma_start(out=wqk_sb[:, :KS // 2, :],
                                    in_=wqk_d[:, :KS // 2, :])
                nc.scalar.dma_start(out=wqk_sb[:, KS // 2:, :],
                                    in_=wqk_d[:, KS // 2:, :])
                nc.scalar.dma_start(out=wv_sb[:], in_=wv_d[:, :, :])

            def layer_norm_stats(parts, name, tagp="", eps=None):
                """parts: list of (tile, ncols512). Returns (nmu, rstd) [P,1]."""
                st = p_ln.tile([P, 4, 6], F32, tag=tagp + "lnst", name=f"st_{name}")
                a = 0
                for tile_, n in parts:
                    for i in range(n):
                        nc.vector.bn_stats(st[:, a, :], tile_[:, 512 * i:512 * (i + 1)])
                        a += 1
                assert a == 4
                mv = p_ln.tile([P, 2], F32, tag=tagp + "lnmv", name=f"mv_{name}")
                nc.vector.bn_aggr(mv[:], st[:])
                lv = p_ln.tile([P, 1], F32, tag=tagp + "lnsd", name=f"lv_{name}")
                nc.scalar.activation(lv[:], mv[:, 1:2],
                                     mybir.ActivationFunctionType.Ln,
                                     bias=(eps if eps is not None else epsb)[:])
                rstd = p_ln.tile([P, 1], F32, tag=tagp + "lnrstd", name=f"rstd_{name}")
                nc.scalar.activation(rstd[:], lv[:],
                                     mybir.ActivationFunctionType.Exp, scale=-0.5)
                nmu = p_ln.tile([P, 1], F32, tag=tagp + "lnnmu", name=f"nmu_{name}")
                nc.vector.tensor_scalar_mul(nmu[:], mv[:, 0:1], -1.0)
                return nmu[:], rstd[:]

            def ln_apply(dst, src, nmu, rstd, engine):
                engine.tensor_scalar(dst, src, nmu, rstd,
                                     mybir.AluOpType.add, mybir.AluOpType.mult)

            # ================= Stage A: LN1, transpose, QKV ===================
            # ksb/qT live in one big tile so the whole region can be reused by
            # the bf16 u tiles of the fused MLP once attention is done.
            kq_all = p_kvu.tile([P, 2, 2, HL, TPB], BF16, tag="kvu", name="kq_all")

            def ksb(b):
                return kq_all[:, b, 0]

            def qT(b):
                return kq_all[:, b, 1]

            vsb = [None, None]
            xpre = {}

            def emit_x_tile(t):
                xh = []
                for hh in range(2):
                    xth = p_x.tile([P, H // 2], BF16, tag="xt",
                                   name=f"xt_{t}_{hh}", bufs=3)
                    (nc.sync if hh == 0 else nc.scalar).dma_start(
                        out=xth[:], in_=x_d[P * t:P * (t + 1),
                                           (H // 2) * hh:(H // 2) * (hh + 1)])
                    xh.append(xth)
                return xh

            def emit_A_group(b, g):
                """LN1 + transpose + QKV for GT=256 tokens (group g of batch b)."""
                if g == 0:
                    vsb[b] = p_vsb.tile([P, QT_PER_B, HL, P + 2], BF16, tag="vsb",
                                        name=f"vsb_{b}")
                    nc.vector.memset(vsb[b][:, :, :, P:P + 1], 1.0)
                hT = p_hT.tile([P, KS, GT], FP8, tag="hT", name=f"hT_{b}_{g}")
                if b == 0:
                    ev_copy = lambda out, in_: nc.scalar.copy(out=out, in_=in_)
                    ev_bias = lambda out, in_, s: nc.scalar.add(out, in_, s)
                else:
                    ev_copy = lambda out, in_: nc.vector.tensor_copy(out=out, in_=in_)
                    ev_bias = lambda out, in_, s: nc.vector.tensor_scalar_add(
                        out, in_, s)
                for tt in range(GT // P):   # 128-token LN tiles
                    t = (TPB * b + GT * g) // P + tt
                    xh = xpre.pop(t, None)
                    if xh is None:
                        xh = emit_x_tile(t)
                    nmu, rstd = layer_norm_stats([(xh[0], 2), (xh[1], 2)],
                                                 f"ln1_{t}")
                    ht = p_h.tile([P, H], BF16, tag="ht", name=f"ht_{t}", bufs=2)
                    for hh in range(2):
                        ln_apply(ht[:, (H // 2) * hh:(H // 2) * (hh + 1)],
                                 xh[hh][:], nmu, rstd, nc.gpsimd)
                    for fg in range(KS // 8):
                        ptp = ps_qs.tile([P, 1024], BF16, tag="psqs",
                                         name=f"trp_{t}_{fg}")
                        for f4 in range(8):
                            f = 8 * fg + f4
                            nc.tensor.transpose(ptp[:, P * f4:P * (f4 + 1)],
                                                ht[:, P * f:P * (f + 1)], ident[:])
                        ev_copy(hT[:, 8 * fg:8 * (fg + 1), P * tt:P * (tt + 1)],
                                ptp[:].rearrange("p (a b) -> p a b", b=P))

                col0 = GT * g
                # QK projection: m 0,1 -> Q head0/1 ; 2,3 -> K head0/1
                for m in range(4):
                    ps = ps_qs.tile([P, GT], F32, tag="psqs", name=f"qk_{b}_{g}_{m}")
                    for k2 in range(KS // 2):
                        nc.tensor.matmul(
                            ps[:], lhsT=wqk_sb[:, 2 * k2:2 * k2 + 2, P * m:P * (m + 1)],
                            rhs=hT[:, 2 * k2:2 * k2 + 2, :],
                            perf_mode=mybir.MatmulPerfMode.DoubleRow,
                            start=(k2 == 0), stop=(k2 == KS // 2 - 1))
                    dst = qT(b) if m < 2 else ksb(b)
                    ev_bias(dst[:, m % 2, col0:col0 + GT], ps[:],
                            bqk_sb[:, m:m + 1])
                # V projection (token-major)
                for m in range(GT // P):
                    ps = ps_sm.tile([P, DV], F32, tag="pssm", name=f"v_{b}_{g}_{m}")
                    for k2 in range(KS // 2):
                        nc.tensor.matmul(
                            ps[:], lhsT=hT[:, 2 * k2:2 * k2 + 2, P * m:P * (m + 1)],
                            rhs=wv_sb[:, 2 * k2:2 * k2 + 2, :],
                            perf_mode=mybir.MatmulPerfMode.DoubleRow,
                            start=(k2 == 0), stop=(k2 == KS // 2 - 1))
                    tm = (GT * g) // P + m
                    nc.vector.tensor_tensor(
                        vsb[b][:, tm, :, 0:P],
                        ps[:].rearrange("p (a b) -> p a b", b=P),
                        bvbc_sb[:], mybir.AluOpType.add)

            # ================= Stage B: attention (S^T form) ==================
            aosb = {}

            def emit_B_S(b, qt, lh):
                """S^T matmuls + mask + exp for (batch, query tile, local head)."""
                klen = P * (qt + 1)
                nchs = (qt + 4) // 4
                ex = p_ex.tile([P, TPB], BF16, tag="ex", name=f"ex_{b}_{qt}_{lh}")
                qcols = qT(b)[:, lh, P * qt:P * (qt + 1)]
                for j in range(nchs):
                    n0 = 512 * j
                    n1 = min(n0 + 512, klen)
                    ps = ps_qs.tile([P, 512], F32, tag="psqs",
                                    name=f"s_{b}_{qt}_{lh}_{j}")
                    for kb in range(n0 // P, n1 // P):
                        nc.tensor.matmul(ps[:, P * kb - n0:P * (kb + 1) - n0],
                                         lhsT=ksb(b)[:, lh, P * kb:P * (kb + 1)],
                                         rhs=qcols, start=True, stop=True)
                    if j == nchs - 1:
                        d0 = klen - P - n0
                        nc.vector.tensor_tensor(ps[:, d0:d0 + P], ps[:, d0:d0 + P],
                                                cmaskT[:], mybir.AluOpType.add)
                    nc.scalar.activation(ex[:, n0:n1], ps[:, :n1 - n0],
                                         mybir.ActivationFunctionType.Exp,
                                         scale=float(SCORE_SCALE))
                return ex

            def emit_B_AV(b, qt, lh, ex):
                """A@V with ones-column, normalize; transpose + fp8-stage after
                lh=1 so the a2a payload is already in O-projection lhsT form."""
                mt = QT_PER_B * b + qt
                if lh == 0:
                    aosb[mt] = p_ao.tile([P, HL, P], BF16, tag="aot", name=f"ao_{mt}")
                psO = ps_sm.tile([P, P + 2], F32, tag="pssm", name=f"o_{mt}_{lh}")
                for kb in range(qt + 1):
                    nc.tensor.matmul(psO[:, :P + 1],
                                     lhsT=ex[:, P * kb:P * (kb + 1)],
                                     rhs=vsb[b][:, kb, lh, 0:P + 1],
                                     start=(kb == 0), stop=(kb == qt))
                rinv = p_ln.tile([P, 1], F32, tag="rinv", name=f"ri_{mt}_{lh}")
                nc.vector.reciprocal(rinv[:], psO[:, P:P + 1])
                # aosb = attn_true (v carries the 16x weight scale; /16 here)
                nc.vector.tensor_scalar(aosb[mt][:, lh, :], psO[:, 0:P],
                                        rinv[:], 1.0 / 16.0,
                                        mybir.AluOpType.mult, mybir.AluOpType.mult)
                if lh == HL - 1:
                    # transpose [tok, 2*128f] -> [2, 128f, tok], cast fp8, stage
                    ptp = ps_sm.tile([P, HL * P], BF16, tag="pssm",
                                     name=f"aop_{mt}")
                    for hh in range(HL):
                        nc.tensor.transpose(ptp[:, P * hh:P * (hh + 1)],
                                            aosb[mt][:, hh, :], ident[:])
                    aoT = p_aoT.tile([P, HL, P], BF16, tag="aoT", name=f"aoT_{mt}")
                    nc.scalar.copy(
                        out=aoT[:],
                        in_=ptp[:].rearrange("p (a b) -> p a b", b=P))
                    g, s, jj = mt // 16, mt % 8, (mt // 8) % 2
                    r0 = 512 * s + 256 * jj
                    nc.sync.dma_start(
                        aot_dram[g, r0:r0 + 256, :].rearrange(
                            "(a p) t -> p a t", a=HL),
                        aoT[:])
                    del aosb[mt]

            rg = [list(range(NCORES))]

            def emit_collective(g):
                nc.gpsimd.collective_compute(
                    "AllToAll", mybir.AluOpType.bypass, replica_groups=rg,
                    ins=[aot_dram[g, :, :]], outs=[a2a_dram[g, :, :]])

            # ================= Stage C: O-proj + LN2 per chunk ================
            h2T = p_ali.tile([P, KS, NCHUNK * P], BF16, tag="ali16", name="h2T")
            h2T8 = p_h2T8.tile([P, KS, NCHUNK * P], FP8, tag="h2T8", name="h2T8")
            x2t = [None] * NCHUNK
            c_state = {}

            def emit_xres(j):
                """xres (host: 16*(x+b2), f32) lands directly in the x2 tile."""
                x2t[j] = p_x2.tile([P, H], F32, tag="x2keep", name=f"x2_{j}")
                nc.scalar.dma_start(x2t[j][:], xres_d[P * j:P * (j + 1), :])

            def emit_C1(j):
                """a2a readback (already fp8 lhsT) + O-proj + scaled residual."""
                g, jj = j // 2, j % 2
                afT = p_afT.tile([P, KS, P], FP8, tag="afT", name=f"afT_{j}")
                a2av = a2a_dram[g].rearrange("(s j f p) t -> p s j f t",
                                             s=NCORES, j=2, p=P)
                for fs in range(2):
                    nc.gpsimd.dma_start(
                        out=afT[:].rearrange("p (s f) t -> p s f t",
                                             s=NCORES)[:, :, fs],
                        in_=a2av[:, :, jj, fs])
                x2 = x2t[j]
                for nn in range(4):
                    psn = ps_c.tile([P, 512], F32, tag="psc", name=f"op_{j}_{nn}")
                    for k2 in range(KS // 2):
                        nc.tensor.matmul(
                            psn[:], lhsT=afT[:, 2 * k2:2 * k2 + 2, :],
                            rhs=wo_sb[:, 2 * k2:2 * k2 + 2, 512 * nn:512 * (nn + 1)],
                            perf_mode=mybir.MatmulPerfMode.DoubleRow,
                            start=(k2 == 0), stop=(k2 == KS // 2 - 1))
                    c0 = 512 * nn
                    # x2 = psO2 + 16*(x+b2)  -> 16 * x2_true   (in place)
                    nc.vector.tensor_tensor(
                        x2[:, c0:c0 + 512], psn[:], x2[:, c0:c0 + 512],
                        mybir.AluOpType.add)
                nmu, rstd = layer_norm_stats([(x2, 4)], f"ln2_{j}", tagp="c2",
                                             eps=epsb2)
                c_state[j] = (x2, nmu, rstd)

            def emit_C2(j):
                """LN2 apply + h2 transpose into h2T (bf16) and h2T8 (fp8)."""
                x2, nmu, rstd = c_state.pop(j)
                h2 = p_h.tile([P, H], BF16, tag="ht", name=f"h2_{j}", bufs=2)
                for hh in range(2):
                    ln_apply(h2[:, (H // 2) * hh:(H // 2) * (hh + 1)],
                             x2[:, (H // 2) * hh:(H // 2) * (hh + 1)],
                             nmu, rstd, nc.gpsimd)
                for fg in range(KS // 8):
                    ptp = ps_c.tile([P, 1024], BF16, tag="psc", name=f"h2t_{j}_{fg}")
                    for f4 in range(8):
                        f = 8 * fg + f4
                        nc.tensor.transpose(ptp[:, P * f4:P * (f4 + 1)],
                                            h2[:, P * f:P * (f + 1)], ident[:])
                    nc.vector.tensor_copy(
                        out=h2T[:, 8 * fg:8 * (fg + 1), P * j:P * (j + 1)],
                        in_=ptp[:].rearrange("p (a b) -> p a b", b=P))
                    nc.scalar.copy(
                        out=h2T8[:, 8 * fg:8 * (fg + 1), P * j:P * (j + 1)],
                        in_=ptp[:].rearrange("p (a b) -> p a b", b=P))

            # ================= Stage D: MLP1 (fused, u stays in SBUF) =========
            silu_fn = (mybir.ActivationFunctionType.Sigmoid if sim
                       else mybir.ActivationFunctionType.Silu)
            # u8p: fp8 mid-tiles 0..N2F-1 as DoubleRow pairs; ubf: tiles N2F..63
            u8p = p_u8.tile([P, N2P, 2, 512], FP8, tag="u8p", name="u8p")
            ubf = None   # allocated after attention frees kq_all

            def u_dst(mm, c0, cn):
                if mm < N2F:
                    return u8p[:, mm // 2, mm % 2, c0:c0 + cn]
                return ubf[:, mm - N2F, c0:c0 + cn]

            def emit_D_tile(mm, c0, cn, w1t=None):
                """MLP1 mid-tile mm over token cols [c0, c0+cn)."""
                fp8 = mm < N1F
                if w1t is None:
                    if fp8:
                        w1t = p_w1.tile([P, KS, P], FP8, tag="w1t",
                                        name=f"w1t_{mm}_{c0}")
                        nc.gpsimd.dma_start(out=w1t[:], in_=w1f8_d[mm, :, :, :])
                    else:
                        w1t = p_w1.tile([P, KS, P], BF16, tag="w1t",
                                        name=f"w1t_{mm}_{c0}")
                        nc.gpsimd.dma_start(out=w1t[:], in_=w1bf_d[mm - N1F, :, :, :])
                if cn == 512:
                    ps = ps_qs.tile([P, 512], F32, tag="psqs", name=f"u_{mm}")
                else:
                    ps = ps_sm.tile([P, cn], F32, tag="pssm", name=f"u_{mm}_{c0}")
                if fp8:
                    for k2 in range(KS // 2):
                        nc.tensor.matmul(
                            ps[:], lhsT=w1t[:, 2 * k2:2 * k2 + 2, :],
                            rhs=h2T8[:, 2 * k2:2 * k2 + 2, c0:c0 + cn],
                            perf_mode=mybir.MatmulPerfMode.DoubleRow,
                            start=(k2 == 0), stop=(k2 == KS // 2 - 1))
                    sc = 1.0 / 16.0
                else:
                    for ks in range(KS):
                        nc.tensor.matmul(ps[:], lhsT=w1t[:, ks, :],
                                         rhs=h2T[:, ks, c0:c0 + cn],
                                         start=(ks == 0), stop=(ks == KS - 1))
                    sc = 1.0
                nc.scalar.activation(u_dst(mm, c0, cn), ps[:], silu_fn,
                                     bias=b1_sb[:, mm:mm + 1], scale=sc)
                return w1t

            # ================= Stage E: MLP2 (4 feature passes) ===============
            def emit_E_pass(p):
                psY = [ps_qs.tile([P, 512], F32, tag="psqs", name=f"y_{p}_{jj}")
                       for jj in range(4)]
                nunit = N2P + NBF2
                for un in range(nunit):
                    if un < N2P:
                        w2t = p_w2.tile([P, 2, 512], FP8, tag="w2t",
                                        name=f"w2t_{p}_{un}")
                        nc.gpsimd.dma_start(out=w2t[:], in_=w2f8_d[p, un, :, :, :])
                        for jj in range(4):
                            nc.tensor.matmul(
                                psY[jj][:],
                                lhsT=u8p[:, un, :, P * jj:P * (jj + 1)],
                                rhs=w2t[:],
                                perf_mode=mybir.MatmulPerfMode.DoubleRow,
                                start=(un == 0), stop=(un == nunit - 1))
                    else:
                        w2t = p_w2.tile([P, 512], BF16, tag="w2t",
                                        name=f"w2t_{p}_{un}")
                        nc.gpsimd.dma_start(out=w2t[:],
                                            in_=w2bf_d[p, un - N2P, :, :])
                        for jj in range(4):
                            nc.tensor.matmul(
                                psY[jj][:],
                                lhsT=ubf[:, un - N2P, P * jj:P * (jj + 1)],
                                rhs=w2t[:],
                                start=(un == 0), stop=(un == nunit - 1))
                for jj in range(4):
                    # out = (psY + 16*x2_true) / 16: add in psum, scaled copy
                    nc.vector.tensor_tensor(
                        psY[jj][:], psY[jj][:], x2t[jj][:, 512 * p:512 * (p + 1)],
                        mybir.AluOpType.add)
                    ot = p_x.tile([P, 512], BF16, tag="xt", name=f"ot_{p}_{jj}", bufs=3)
                    nc.scalar.activation(ot[:], psY[jj][:],
                                         mybir.ActivationFunctionType.Copy,
                                         scale=1.0 / X2S)
                    nc.scalar.dma_start(
                        out=out_d[P * jj:P * (jj + 1), 512 * p:512 * (p + 1)],
                        in_=ot[:])

            # ================= emission schedule ==============================
            for t in range(2):
                xpre[t] = emit_x_tile(t)
            emit_weight_dmas()
            for g in range(NG_PER_B):
                for tt in range(2):
                    tn = 2 * (g + 1) + tt
                    if tn < 16:
                        xpre[tn] = emit_x_tile(tn)
                emit_A_group(0, g)

            # attention(b0) interleaved with QKV(b1)
            for qt in range(QT_PER_B):
                if qt % 2 == 0:
                    for tt in range(2):
                        xpre[16 + qt + tt] = emit_x_tile(16 + qt + tt)
                else:
                    emit_A_group(1, qt // 2)
                exs = [emit_B_S(0, qt, lh) for lh in range(HL)]
                for lh in range(HL):
                    emit_B_AV(0, qt, lh, exs[lh])
                if qt == 1:
                    nc.scalar.dma_start(out=wo_sb[:], in_=wo_d[:, :, :])
                if qt == 3:
                    for j in range(NCHUNK):
                        emit_xres(j)
            emit_collective(0)

            # attention(b1) interleaved with chunk 0/1 post-processing
            for qt in range(QT_PER_B - 1, -1, -1):
                exs = [emit_B_S(1, qt, lh) for lh in range(HL)]
                for lh in range(HL):
                    emit_B_AV(1, qt, lh, exs[lh])
                if qt == 7:
                    emit_C1(0)
                if qt == 6:
                    emit_C2(0)
                if qt == 5:
                    emit_C1(1)
                if qt == 4:
                    emit_C2(1)
            emit_collective(1)

            # early MLP1: fp8 tiles, first token half (chunks 0,1) -- fills the
            # AllToAll latency window; their W1 tiles are streamed again for
            # the second half (cheap: 5MB fp8)
            for mm in range(N1F):
                emit_D_tile(mm, 0, 256)
            emit_C1(2)
            emit_C2(2)
            emit_C1(3)
            emit_C2(3)
            # now the kq region is dead (attention complete) -> bf16 u tiles
            ubf = p_kvu.tile([P, MMT - N2F, 512], BF16, tag="kvu", name="ubf")
            for mm in range(N1F):
                emit_D_tile(mm, 256, 256)
            for mm in range(N1F, MMT):
                emit_D_tile(mm, 0, 512)
            for p in range(NPASS):
                emit_E_pass(p)
    nc.compile()
    return nc


def _bf16(a):
    return np.asarray(a, dtype=np.float32).astype(ml_dtypes.bfloat16)


def _fp8(a):
    return np.clip(np.asarray(a, np.float32), -240, 240).astype(mybir.dt.np(FP8))


def make_in_maps(x, Wq, Wk, Wv, Wo, g1, bn1, g2, bn2, W1, b1, W2, b2):
    x = np.asarray(x, np.float32)
    x_flat = np.ascontiguousarray(x.reshape(NTOK, H))

    wq_eff = (g1[:, None] * np.asarray(Wq, np.float32)) * WSCALE
    wk_eff = (g1[:, None] * np.asarray(Wk, np.float32)) * WSCALE
    wv_eff = (g1[:, None] * np.asarray(Wv, np.float32)) * WSCALE
    bq = (bn1 @ np.asarray(Wq, np.float32)) * WSCALE
    bk = (bn1 @ np.asarray(Wk, np.float32)) * WSCALE
    bv = (bn1 @ np.asarray(Wv, np.float32)) * WSCALE
    w1_eff = g2[:, None] * np.asarray(W1, np.float32)
    b1_eff = np.asarray(b1, np.float32) + bn2 @ np.asarray(W1, np.float32)

    xbf = np.ascontiguousarray(_bf16(x_flat))
    # W1: [mm, p, ks, mw]; tiles 0..N1F-1 fp8 (x16), rest bf16
    w1_t = _bf16(w1_eff).astype(np.float32).reshape(KS, P, MMT, P).transpose(2, 1, 0, 3)
    w1f8 = np.ascontiguousarray(_fp8(16.0 * w1_eff.reshape(KS, P, MMT, P)
                                     .transpose(2, 1, 0, 3)[:N1F]))
    w1bf = np.ascontiguousarray(_bf16(w1_eff.reshape(KS, P, MMT, P)
                                      .transpose(2, 1, 0, 3)[N1F:]))
    # W2 scaled by 32 on both dtypes; [pass][unit][...]
    W2f = np.asarray(W2, np.float32) * 16.0
    w2f8 = np.empty((NPASS, N2P, P, 2, 512), mybir.dt.np(FP8))
    w2bf = np.empty((NPASS, NBF2, P, 512), ml_dtypes.bfloat16)
    for p in range(NPASS):
        cols = slice(512 * p, 512 * (p + 1))
        for q in range(N2P):
            w2f8[p, q, :, 0, :] = _fp8(W2f[P * 2 * q:P * (2 * q + 1), cols])
            w2f8[p, q, :, 1, :] = _fp8(W2f[P * (2 * q + 1):P * (2 * q + 2), cols])
        for i in range(NBF2):
            mm = N2F + i
            w2bf[p, i] = _bf16(W2f[P * mm:P * (mm + 1), cols])
    b1m = np.ascontiguousarray(b1_eff.reshape(MMT, P).T.astype(np.float32))
    wo8 = np.ascontiguousarray(
        _fp8(16.0 * np.asarray(Wo, np.float32)).reshape(KS, P, H).transpose(1, 0, 2))
    ii, jj_ = np.meshgrid(np.arange(P), np.arange(P), indexing="ij")
    cmaskT = np.where(ii <= jj_, 0.0, NEG).astype(np.float32)
    b2f = np.asarray(b2, np.float32)

    in_maps = []
    for c in range(NCORES):
        cs = slice(DV * c, DV * (c + 1))
        wqk = np.concatenate([wq_eff[:, cs], wk_eff[:, cs]], axis=1)  # [H, 512]
        wqk_t = np.ascontiguousarray(
            _fp8(wqk).reshape(KS, P, DQK).transpose(1, 0, 2))
        bqk = np.concatenate([bq[cs], bk[cs]]).astype(np.float32)
        bqk_m = np.ascontiguousarray(bqk.reshape(DQK // P, P).T)
        wv_t = np.ascontiguousarray(
            _fp8(wv_eff[:, cs]).reshape(KS, P, DV).transpose(1, 0, 2))
        bvbc = np.ascontiguousarray(np.broadcast_to(
            bv[cs].astype(np.float32).reshape(1, HL, P), (P, HL, P)))
        xres = np.concatenate(
            [x_flat[1024 * j + P * c:1024 * j + P * (c + 1)] for j in range(NCHUNK)],
            axis=0) + b2f
        xres16 = np.ascontiguousarray((16.0 * xres).astype(np.float32))
        in_maps.append({
            "xbf": xbf, "xres": xres16,
            "wqk": wqk_t, "bqk": bqk_m, "wv": wv_t, "bvbc": bvbc, "wo": wo8,
            "w1f8": w1f8, "w1bf": w1bf, "b1": b1m, "w2f8": w2f8, "w2bf": w2bf,
            "cmaskT": cmaskT,
        })
    return in_maps


_NC_CACHE = {}


def kernel(**inputs):
    if "nc" not in _NC_CACHE:
        _NC_CACHE["nc"] = build()
    nc = _NC_CACHE["nc"]
    in_maps = make_in_maps(
        inputs["x"], inputs["Wq"], inputs["Wk"], inputs["Wv"], inputs["Wo"],
        np.asarray(inputs["g1"], np.float32), np.asarray(inputs["bn1"], np.float32),
        np.asarray(inputs["g2"], np.float32), np.asarray(inputs["bn2"], np.float32),
        inputs["W1"], inputs["b1"], inputs["W2"], inputs["b2"])
    res = run_bass_kernel_spmd(nc, in_maps, list(range(NCORES)))
    out = np.empty((NTOK, H), np.float32)
    for c in range(NCORES):
        oc = np.asarray(res.results[c]["out"], dtype=np.float32)
        for j in range(NCHUNK):
            out[1024 * j + P * c:1024 * j + P * (c + 1)] = oc[P * j:P * (j + 1)]
    return out.reshape(B, T, H)


# revision 18
# speedup vs baseline: 1.2222x; 1.2222x over previous
"""Fused transformer block (LN -> causal MHA -> residual -> LN -> SiLU MLP -> residual)
on 8 Trainium2 NeuronCores.

v3 design (on top of the v2 baseline):
- Tensor-parallel over heads (2 heads/core) for QKV + attention; S computed
  transposed (S^T) with the softmax denominator as a ones-column of V.
- Attention outputs are TRANSPOSED and quantized to fp8 BEFORE the AllToAll
  (payload halves to 512KB/group; after the exchange the received buffer is
  directly the lhsT of the O-projection -> no post-collective PE work beyond
  the matmuls themselves).
- x2 (attention residual) is kept in SBUF in f32, scaled by 32 so that the
  fp8(32*W2) and bf16(32*W2) MLP2 products accumulate uniformly; the final
  drain rescales by 1/32. LN2 is scale-invariant (eps scaled to match).
- MLP is FUSED: u = silu(mlp1) stays in SBUF (aliased onto the dead K/Q
  SBUF region); no DRAM round trip. MLP2 runs in 4 feature passes of 4 PSUM
  banks each.
- Mixed-precision MLP: mid-tiles 0..19 of MLP1 and mid-tile pairs 0..10 of
  MLP2 run fp8+DoubleRow (1.8x); the rest bf16. Chosen so the predicted
  rel-err (numpy-emulated, matches HW to 3 digits) is ~1.8e-2 < 2e-2.
- The first 20 MLP1 tiles run as two 256-token halves so their first halves
  (token chunks 0,1, available right after C2(1)) fill the ~50us AllToAll #2
  latency window; their fp8 W1 tiles are streamed twice (5MB extra).
- PSUM ring decoupling: attention (ring qs/sm), C (own ring c), so the
  collective-dependent O-projection never blocks attention PSUM reuse.
- Output written bf16 (host upcasts), xres folded with b2 and pre-scaled on
  the host.
"""

import sys
import os

for _p in ("/opt/trn_rl_repo", "/root/.axon_site/_ro/trn_rl_repo"):
    if os.path.isdir(_p) and _p not in sys.path:
        sys.path.insert(0, _p)
        break

import numpy as np
import ml_dtypes

import concourse.bass as bass
from concourse import bacc
import concourse.mybir as mybir
import concourse.tile as tile
from concourse.masks import make_identity
from concourse.bass_utils import run_bass_kernel_spmd

F32 = mybir.dt.float32
BF16 = mybir.dt.bfloat16
FP8 = mybir.dt.float8e4


def _install_act_table_hint():
    """Steer the act-table-set chooser so Exp and Ln resolve to the one set
    that contains BOTH (natural_log_exp_and_others)."""
    import concourse.bacc as _bacc
    if getattr(_bacc, "_act_hint_installed", False):
        return
    _orig = _bacc.get_activation_tables

    def _patched(arch):
        tabs = _orig(arch)
        exp = mybir.ActivationFunctionType.Exp
        ln = mybir.ActivationFunctionType.Ln
        for name, fns in tabs.items():
            if name != "natural_log_exp_and_others":
                fns.discard(exp)
                fns.discard(ln)
        return tabs

    _bacc.get_activation_tables = _patched
    _bacc._act_hint_installed = True


_install_act_table_hint()

P = 128          # partitions / head_dim / token tile
H = 2048         # hidden
KS = H // P      # 16 k-subtiles over hidden
HEADS = 16
HL = 2           # heads per core
NCORES = 8
B = 2
T = 2048
NTOK = B * T     # 4096
TPB = T          # tokens per batch
MID = 4 * H      # 8192
MMT = MID // P   # 64 m-tiles over mid dim
DQK = 2 * HL * P   # 512 rows of fused QK projection per core
DV = HL * P        # 256 V/attention-out features per core
EPS = 1e-5
NEG = -1.0e30

QT_PER_B = TPB // P   # 16 q tiles per batch
MT = NTOK // P        # 32 token m-tiles
NCHUNK = 4            # token chunks per core (128 each)
GT = 256              # tokens per A-group
WSCALE = 16.0         # fp8 weight rescale (avoids e4m3 subnormals)
SCORE_SCALE = (1.0 / np.sqrt(P)) / (WSCALE * WSCALE)
NG_PER_B = TPB // GT  # 8 A-groups per batch

# ---- mixed-precision MLP config ----
N1F = 20              # MLP1 mid-tiles 0..N1F-1 in fp8 DoubleRow (also the
                      # "early" tiles run as two 256-token halves)
N2P = 11              # MLP2 mid-tile PAIRS 0..N2P-1 (tiles 0..21) in fp8 DR
N2F = 2 * N2P         # fp8 MLP2 tiles
NBF2 = MMT - N2F      # 42 bf16 MLP2 tiles
NPASS = 4             # MLP2 feature passes (512 cols each)
X2S = 16.0            # x2 kept as 16*x2_true in SBUF


def build(sim=False, trn_kwargs=None, trace_sim=False):
    nc = bacc.Bacc(None, num_devices=NCORES, **(trn_kwargs or {}))

    x_d = nc.declare_dram_parameter("xbf", [NTOK, H], BF16, isOutput=False)
    xres_d = nc.declare_dram_parameter("xres", [NCHUNK * P, H], F32, isOutput=False)
    wqk_d = nc.declare_dram_parameter("wqk", [P, KS, DQK], FP8, isOutput=False)
    bqk_d = nc.declare_dram_parameter("bqk", [P, DQK // P], F32, isOutput=False)
    wv_d = nc.declare_dram_parameter("wv", [P, KS, DV], FP8, isOutput=False)
    bvbc_d = nc.declare_dram_parameter("bvbc", [P, HL, P], F32, isOutput=False)
    wo_d = nc.declare_dram_parameter("wo", [P, KS, H], FP8, isOutput=False)
    w1f8_d = nc.declare_dram_parameter("w1f8", [N1F, P, KS, P], FP8, isOutput=False)
    w1bf_d = nc.declare_dram_parameter("w1bf", [MMT - N1F, P, KS, P], BF16,
                                       isOutput=False)
    b1_d = nc.declare_dram_parameter("b1", [P, MMT], F32, isOutput=False)
    w2f8_d = nc.declare_dram_parameter("w2f8", [NPASS, N2P, P, 2, 512], FP8,
                                       isOutput=False)
    w2bf_d = nc.declare_dram_parameter("w2bf", [NPASS, NBF2, P, 512], BF16,
                                       isOutput=False)
    cmaskT_d = nc.declare_dram_parameter("cmaskT", [P, P], F32, isOutput=False)
    out_d = nc.declare_dram_parameter("out", [NCHUNK * P, H], BF16, isOutput=True)

    from contextlib import ExitStack
    with tile.TileContext(nc, trace_sim=trace_sim) as tc:
        with ExitStack() as stack:
            dram = stack.enter_context(tc.tile_pool(name="dram", bufs=1, space="DRAM"))
            const = stack.enter_context(tc.tile_pool(name="const", bufs=1))
            wbig = stack.enter_context(tc.tile_pool(name="wbig", bufs=1))
            # wqk (8KB/part, dead after QKV) chained with h2T (16KB)
            p_ali = stack.enter_context(tc.tile_pool(name="ali16", bufs=1))
            # ksb+qT (32KB, dead after last AV) chained with ubf (42KB)
            p_kvu = stack.enter_context(tc.tile_pool(name="kvu", bufs=1))
            p_vsb = stack.enter_context(tc.tile_pool(name="vsb", bufs=2))
            p_u8 = stack.enter_context(tc.tile_pool(name="u8", bufs=1))
            p_x = stack.enter_context(tc.tile_pool(name="xin", bufs=2))
            p_ln = stack.enter_context(tc.tile_pool(name="lnsmall", bufs=2))
            p_h = stack.enter_context(tc.tile_pool(name="htok", bufs=2))
            p_hT = stack.enter_context(tc.tile_pool(name="hT", bufs=2))
            p_h2T8 = stack.enter_context(tc.tile_pool(name="h2T8", bufs=1))
            p_ex = stack.enter_context(tc.tile_pool(name="expT", bufs=2))
            p_ao = stack.enter_context(tc.tile_pool(name="aot", bufs=2))
            p_aoT = stack.enter_context(tc.tile_pool(name="aoT", bufs=1))
            p_afT = stack.enter_context(tc.tile_pool(name="afT", bufs=1))
            p_x2 = stack.enter_context(tc.tile_pool(name="x2", bufs=4))
            p_w1 = stack.enter_context(tc.tile_pool(name="w1pool", bufs=2))
            p_w2 = stack.enter_context(tc.tile_pool(name="w2pool", bufs=2))
            # PSUM rings:
            #  qs: 4 x 2KB  (A-QK, A-transposes, B-S, D-full psU, E-psY)
            #  sm: 4 x 1KB  (A-V, B-AV, B-aoT transposes, D-early psU halves)
            #  c : 2 x 2KB  (C O-proj, C2 h2 transposes) -- collective-coupled
            ps_qs = stack.enter_context(tc.tile_pool(name="psqs", bufs=4, space="PSUM"))
            ps_sm = stack.enter_context(tc.tile_pool(name="pssm", bufs=2, space="PSUM"))
            ps_c = stack.enter_context(tc.tile_pool(name="psc", bufs=2, space="PSUM"))

            # ---- internal DRAM ----
            # aotT laid [g][dst s][jj][fsub][f][t]; per-(g,s) shard contiguous
            aot_dram = dram.tile([2, NCORES * 2 * 2 * P, P], BF16)
            a2a_dram = dram.tile([2, NCORES * 2 * 2 * P, P], BF16)

            # ---- constants / weights in SBUF ----
            ident = const.tile([P, P], BF16)
            make_identity(nc, ident)
            epsb = const.tile([P, 1], F32)
            nc.vector.memset(epsb[:], EPS)
            epsb2 = const.tile([P, 1], F32)
            nc.vector.memset(epsb2[:], EPS * X2S * X2S)
            scrap = const.tile([P, 1], F32)
            cmaskT = const.tile([P, P], F32)
            nc.sync.dma_start(cmaskT[:], cmaskT_d[:, :])
            bqk_sb = const.tile([P, DQK // P], F32)
            nc.sync.dma_start(bqk_sb[:], bqk_d[:, :])
            bvbc_sb = const.tile([P, HL, P], F32)
            nc.sync.dma_start(bvbc_sb[:], bvbc_d[:, :, :])
            b1_sb = const.tile([P, MMT], F32)
            nc.sync.dma_start(b1_sb[:], b1_d[:, :])
            wqk_sb = p_ali.tile([P, KS, DQK], FP8, tag="ali16", name="wqk_sb")
            wv_sb = wbig.tile([P, KS, DV], FP8)
            wo_sb = wbig.tile([P, KS, H], FP8)

            def emit_weight_dmas():
                nc.gpsimd.dma_start(out=wqk_sb[:, :KS // 2, :],
                                    in_=wqk_d[:, :KS // 2, :])
                nc.scalar.dma_start(out=wqk_sb[:, KS // 2:, :],
                                    in_=wqk_d[:, KS // 2:, :])
                nc.scalar.dma_start(out=wv_sb[:], in_=wv_d[:, :, :])

            def layer_norm_stats(parts, name, tagp="", eps=None):
                """parts: list of (tile, ncols512). Returns (nmu, rstd) [P,1]."""
                st = p_ln.tile([P, 4, 6], F32, tag=tagp + "lnst", name=f"st_{name}")
                a = 0
                for tile_, n in parts:
                    for i in range(n):
                        nc.vector.bn_stats(st[:, a, :], tile_[:, 512 * i:512 * (i + 1)])
                        a += 1
                assert a == 4
                mv = p_ln.tile([P, 2], F32, tag=tagp + "lnmv", name=f"mv_{name}")
                nc.vector.bn_aggr(mv[:], st[:])
                lv = p_ln.tile([P, 1], F32, tag=tagp + "lnsd", name=f"lv_{name}")
                nc.scalar.activation(lv[:], mv[:, 1:2],
                                     mybir.ActivationFunctionType.Ln,
                                     bias=(eps if eps is not None else epsb)[:])
                rstd = p_ln.tile([P, 1], F32, tag=tagp + "lnrstd", name=f"rstd_{name}")
                nc.scalar.activation(rstd[:], lv[:],
                                     mybir.ActivationFunctionType.Exp, scale=-0.5)
                nmu = p_ln.tile([P, 1], F32, tag=tagp + "lnnmu", name=f"nmu_{name}")
                nc.vector.tensor_scalar_mul(nmu[:], mv[:, 0:1], -1.0)
                return nmu[:], rstd[:]

            def ln_apply(dst, src, nmu, rstd, engine):
                engine.tensor_scalar(dst, src, nmu, rstd,
                                     mybir.AluOpType.add, mybir.AluOpType.mult)

            # ================= Stage A: LN1, transpose, QKV ===================
            # ksb/qT live in one big tile so the whole region can be reused by
            # the bf16 u tiles of the fused MLP once attention is done.
            kq_all = p_kvu.tile([P, 2, 2, HL, TPB], BF16, tag="kvu", name="kq_all")

            def ksb(b):
                return kq_all[:, b, 0]

            def qT(b):
                return kq_all[:, b, 1]

            vsb = [None, None]
            xpre = {}

            def emit_x_tile(t):
                xh = []
                for hh in range(2):
                    xth = p_x.tile([P, H // 2], BF16, tag="xt",
                                   name=f"xt_{t}_{hh}", bufs=3)
                    (nc.sync if hh == 0 else nc.scalar).dma_start(
                        out=xth[:], in_=x_d[P * t:P * (t + 1),
                                           (H // 2) * hh:(H // 2) * (hh + 1)])
                    xh.append(xth)
                return xh

            def emit_A_group(b, g):
                """LN1 + transpose + QKV for GT=256 tokens (group g of batch b)."""
                if g == 0:
                    vsb[b] = p_vsb.tile([P, QT_PER_B, HL, P + 2], BF16, tag="vsb",
                                        name=f"vsb_{b}")
                    nc.vector.memset(vsb[b][:, :, :, P:P + 1], 1.0)
                hT = p_hT.tile([P, KS, GT], FP8, tag="hT", name=f"hT_{b}_{g}")
                if b == 0:
                    ev_copy = lambda out, in_: nc.scalar.copy(out=out, in_=in_)
                    ev_bias = lambda out, in_, s: nc.scalar.add(out, in_, s)
                else:
                    ev_copy = lambda out, in_: nc.vector.tensor_copy(out=out, in_=in_)
                    ev_bias = lambda out, in_, s: nc.vector.tensor_scalar_add(
                        out, in_, s)
                for tt in range(GT // P):   # 128-token LN tiles
                    t = (TPB * b + GT * g) // P + tt
                    xh = xpre.pop(t, None)
                    if xh is None:
                        xh = emit_x_tile(t)
                    nmu, rstd = layer_norm_stats([(xh[0], 2), (xh[1], 2)],
                                                 f"ln1_{t}")
                    ht = p_h.tile([P, H], BF16, tag="ht", name=f"ht_{t}", bufs=2)
                    for hh in range(2):
                        ln_apply(ht[:, (H // 2) * hh:(H // 2) * (hh + 1)],
                                 xh[hh][:], nmu, rstd, nc.gpsimd)
                    for fg in range(KS // 8):
                        ptp = ps_c.tile([P, 1024], BF16, tag="psc",
                                        name=f"trp_{t}_{fg}")
                        for f4 in range(8):
                            f = 8 * fg + f4
                            nc.tensor.transpose(ptp[:, P * f4:P * (f4 + 1)],
                                                ht[:, P * f:P * (f + 1)], ident[:])
                        ev_copy(hT[:, 8 * fg:8 * (fg + 1), P * tt:P * (tt + 1)],
                                ptp[:].rearrange("p (a b) -> p a b", b=P))

                col0 = GT * g
                # QK projection: m 0,1 -> Q head0/1 ; 2,3 -> K head0/1
                for m in range(4):
                    ps = ps_qs.tile([P, GT], F32, tag="psqs", name=f"qk_{b}_{g}_{m}")
                    for k2 in range(KS // 2):
                        nc.tensor.matmul(
                            ps[:], lhsT=wqk_sb[:, 2 * k2:2 * k2 + 2, P * m:P * (m + 1)],
                            rhs=hT[:, 2 * k2:2 * k2 + 2, :],
                            perf_mode=mybir.MatmulPerfMode.DoubleRow,
                            start=(k2 == 0), stop=(k2 == KS // 2 - 1))
                    dst = qT(b) if m < 2 else ksb(b)
                    ev_bias(dst[:, m % 2, col0:col0 + GT], ps[:],
                            bqk_sb[:, m:m + 1])
                # V projection (token-major)
                for m in range(GT // P):
                    ps = ps_sm.tile([P, DV], F32, tag="pssm", name=f"v_{b}_{g}_{m}")
                    for k2 in range(KS // 2):
                        nc.tensor.matmul(
                            ps[:], lhsT=hT[:, 2 * k2:2 * k2 + 2, P * m:P * (m + 1)],
                            rhs=wv_sb[:, 2 * k2:2 * k2 + 2, :],
                            perf_mode=mybir.MatmulPerfMode.DoubleRow,
                            start=(k2 == 0), stop=(k2 == KS // 2 - 1))
                    tm = (GT * g) // P + m
                    nc.vector.tensor_tensor(
                        vsb[b][:, tm, :, 0:P],
                        ps[:].rearrange("p (a b) -> p a b", b=P),
                        bvbc_sb[:], mybir.AluOpType.add)

            # ================= Stage B: attention (S^T form) ==================
            aosb = {}

            def emit_B_S(b, qt, lh):
                """S^T matmuls + mask + exp for (batch, query tile, local head)."""
                klen = P * (qt + 1)
                nchs = (qt + 4) // 4
                ex = p_ex.tile([P, TPB], BF16, tag="ex", name=f"ex_{b}_{qt}_{lh}")
                qcols = qT(b)[:, lh, P * qt:P * (qt + 1)]
                for j in range(nchs):
                    n0 = 512 * j
                    n1 = min(n0 + 512, klen)
                    ps = ps_qs.tile([P, 512], F32, tag="psqs",
                                    name=f"s_{b}_{qt}_{lh}_{j}")
                    for kb in range(n0 // P, n1 // P):
                        nc.tensor.matmul(ps[:, P * kb - n0:P * (kb + 1) - n0],
                                         lhsT=ksb(b)[:, lh, P * kb:P * (kb + 1)],
                                         rhs=qcols, start=True, stop=True)
                    if j == nchs - 1:
                        d0 = klen - P - n0
                        nc.vector.tensor_tensor(ps[:, d0:d0 + P], ps[:, d0:d0 + P],
                                                cmaskT[:], mybir.AluOpType.add)
                    nc.scalar.activation(ex[:, n0:n1], ps[:, :n1 - n0],
                                         mybir.ActivationFunctionType.Exp,
                                         scale=float(SCORE_SCALE))
                return ex

            def emit_B_AV(b, qt, lh, ex):
                """A@V with ones-column, normalize; transpose + fp8-stage after
                lh=1 so the a2a payload is already in O-projection lhsT form."""
                mt = QT_PER_B * b + qt
                if lh == 0:
                    aosb[mt] = p_ao.tile([P, HL, P], BF16, tag="aot", name=f"ao_{mt}")
                psO = ps_sm.tile([P, P + 2], F32, tag="pssm", name=f"o_{mt}_{lh}")
                for kb in range(qt + 1):
                    nc.tensor.matmul(psO[:, :P + 1],
                                     lhsT=ex[:, P * kb:P * (kb + 1)],
                                     rhs=vsb[b][:, kb, lh, 0:P + 1],
                                     start=(kb == 0), stop=(kb == qt))
                rinv = p_ln.tile([P, 1], F32, tag="rinv", name=f"ri_{mt}_{lh}")
                nc.vector.reciprocal(rinv[:], psO[:, P:P + 1])
                # aosb = attn_true (v carries the 16x weight scale; /16 here)
                nc.vector.tensor_scalar(aosb[mt][:, lh, :], psO[:, 0:P],
                                        rinv[:], 1.0 / 16.0,
                                        mybir.AluOpType.mult, mybir.AluOpType.mult)
                if lh == HL - 1:
                    # transpose [tok, 2*128f] -> [2, 128f, tok], cast fp8, stage
                    ptp = ps_sm.tile([P, HL * P], BF16, tag="pssm",
                                     name=f"aop_{mt}")
                    for hh in range(HL):
                        nc.tensor.transpose(ptp[:, P * hh:P * (hh + 1)],
                                            aosb[mt][:, hh, :], ident[:])
                    aoT = p_aoT.tile([P, HL, P], BF16, tag="aoT", name=f"aoT_{mt}")
                    nc.scalar.copy(
                        out=aoT[:],
                        in_=ptp[:].rearrange("p (a b) -> p a b", b=P))
                    g, s, jj = mt // 16, mt % 8, (mt // 8) % 2
                    r0 = 512 * s + 256 * jj
                    nc.sync.dma_start(
                        aot_dram[g, r0:r0 + 256, :].rearrange(
                            "(a p) t -> p a t", a=HL),
                        aoT[:])
                    del aosb[mt]

            rg = [list(range(NCORES))]

            def emit_collective(g):
                nc.gpsimd.collective_compute(
                    "AllToAll", mybir.AluOpType.bypass, replica_groups=rg,
                    ins=[aot_dram[g, :, :]], outs=[a2a_dram[g, :, :]])

            # ================= Stage C: O-proj + LN2 per chunk ================
            h2T = p_ali.tile([P, KS, NCHUNK * P], BF16, tag="ali16", name="h2T")
            h2T8 = p_h2T8.tile([P, KS, NCHUNK * P], FP8, tag="h2T8", name="h2T8")
            x2t = [None] * NCHUNK
            c_state = {}

            def emit_xres(j):
                """xres (host: 16*(x+b2), f32) lands directly in the x2 tile."""
                x2t[j] = p_x2.tile([P, H], F32, tag="x2keep", name=f"x2_{j}")
                nc.scalar.dma_start(x2t[j][:], xres_d[P * j:P * (j + 1), :])

            def emit_C1(j):
                """a2a readback (already fp8 lhsT) + O-proj + scaled residual."""
                g, jj = j // 2, j % 2
                afT = p_afT.tile([P, KS, P], FP8, tag="afT", name=f"afT_{j}")
                a2av = a2a_dram[g].rearrange("(s j f p) t -> p s j f t",
                                             s=NCORES, j=2, p=P)
                for fs in range(2):
                    nc.gpsimd.dma_start(
                        out=afT[:].rearrange("p (s f) t -> p s f t",
                                             s=NCORES)[:, :, fs],
                        in_=a2av[:, :, jj, fs])
                x2 = x2t[j]
                for nn in range(4):
                    psn = ps_c.tile([P, 512], F32, tag="psc", name=f"op_{j}_{nn}")
                    for k2 in range(KS // 2):
                        nc.tensor.matmul(
                            psn[:], lhsT=afT[:, 2 * k2:2 * k2 + 2, :],
                            rhs=wo_sb[:, 2 * k2:2 * k2 + 2, 512 * nn:512 * (nn + 1)],
                            perf_mode=mybir.MatmulPerfMode.DoubleRow,
                            start=(k2 == 0), stop=(k2 == KS // 2 - 1))
                    c0 = 512 * nn
                    # x2 = psO2 + 16*(x+b2)  -> 16 * x2_true   (in place)
                    nc.vector.tensor_tensor(
                        x2[:, c0:c0 + 512], psn[:], x2[:, c0:c0 + 512],
                        mybir.AluOpType.add)
                nmu, rstd = layer_norm_stats([(x2, 4)], f"ln2_{j}", tagp="c2",
                                             eps=epsb2)
                c_state[j] = (x2, nmu, rstd)

            def emit_C2(j):
                """LN2 apply + h2 transpose into h2T (bf16) and h2T8 (fp8)."""
                x2, nmu, rstd = c_state.pop(j)
                h2 = p_h.tile([P, H], BF16, tag="ht", name=f"h2_{j}", bufs=2)
                for hh in range(2):
                    ln_apply(h2[:, (H // 2) * hh:(H // 2) * (hh + 1)],
                             x2[:, (H // 2) * hh:(H // 2) * (hh + 1)],
                             nmu, rstd, nc.gpsimd)
                for fg in range(KS // 8):
                    ptp = ps_c.tile([P, 1024], BF16, tag="psc", name=f"h2t_{j}_{fg}")
                    for f4 in range(8):
                        f = 8 * fg + f4
                        nc.tensor.transpose(ptp[:, P * f4:P * (f4 + 1)],
                                            h2[:, P * f:P * (f + 1)], ident[:])
                    nc.vector.tensor_copy(
                        out=h2T[:, 8 * fg:8 * (fg + 1), P * j:P * (j + 1)],
                        in_=ptp[:].rearrange("p (a b) -> p a b", b=P))
                    nc.scalar.copy(
                        out=h2T8[:, 8 * fg:8 * (fg + 1), P * j:P * (j + 1)],
                        in_=ptp[:].rearrange("p (a b) -> p a b", b=P))

            # ================= Stage D: MLP1 (fused, u stays in SBUF) =========
            silu_fn = (mybir.ActivationFunctionType.Sigmoid if sim
                       else mybir.ActivationFunctionType.Silu)
            # u8p: fp8 mid-tiles 0..N2F-1 as DoubleRow pairs; ubf: tiles N2F..63
            u8p = p_u8.tile([P, N2P, 2, 512], FP8, tag="u8p", name="u8p")
            ubf = None   # allocated after attention frees kq_all

            def u_dst(mm, c0, cn):
                if mm < N2F:
                    return u8p[:, mm // 2, mm % 2, c0:c0 + cn]
                return ubf[:, mm - N2F, c0:c0 + cn]

            def emit_D_tile(mm, c0, cn, w1t=None):
                """MLP1 mid-tile mm over token cols [c0, c0+cn)."""
                fp8 = mm < N1F
                if w1t is None:
                    pool, tg = ((p_w1, "w1t") if mm % 2 == 0 else (p_hT, "hT"))
                    q = nc.gpsimd if mm % 2 == 0 else nc.sync
                    if fp8:
                        w1t = pool.tile([P, KS, P], FP8, tag=tg,
                                        name=f"w1t_{mm}_{c0}")
                        q.dma_start(out=w1t[:], in_=w1f8_d[mm, :, :, :])
                    else:
                        w1t = pool.tile([P, KS, P], BF16, tag=tg,
                                        name=f"w1t_{mm}_{c0}")
                        q.dma_start(out=w1t[:], in_=w1bf_d[mm - N1F, :, :, :])
                if cn == 512:
                    ps = ps_qs.tile([P, 512], F32, tag="psqs", name=f"u_{mm}")
                else:
                    ps = ps_sm.tile([P, cn], F32, tag="pssm", name=f"u_{mm}_{c0}")
                if fp8:
                    for k2 in range(KS // 2):
                        nc.tensor.matmul(
                            ps[:], lhsT=w1t[:, 2 * k2:2 * k2 + 2, :],
                            rhs=h2T8[:, 2 * k2:2 * k2 + 2, c0:c0 + cn],
                            perf_mode=mybir.MatmulPerfMode.DoubleRow,
                            start=(k2 == 0), stop=(k2 == KS // 2 - 1))
                    sc = 1.0 / 16.0
                else:
                    for ks in range(KS):
                        nc.tensor.matmul(ps[:], lhsT=w1t[:, ks, :],
                                         rhs=h2T[:, ks, c0:c0 + cn],
                                         start=(ks == 0), stop=(ks == KS - 1))
                    sc = 1.0
                nc.scalar.activation(u_dst(mm, c0, cn), ps[:], silu_fn,
                                     bias=b1_sb[:, mm:mm + 1], scale=sc)
                return w1t

            # ================= Stage E: MLP2 (4 feature passes) ===============
            def emit_E_pass(p):
                psY = [ps_qs.tile([P, 512], F32, tag="psqs", name=f"y_{p}_{jj}")
                       for jj in range(4)]
                nunit = N2P + NBF2
                for un in range(nunit):
                    pool, tg = ((p_w2, "w2t") if un % 2 == 0 else (p_vsb, "vsb"))
                    q = nc.sync if un % 2 == 0 else nc.scalar
                    if un < N2P:
                        w2t = pool.tile([P, 2, 512], FP8, tag=tg,
                                        name=f"w2t_{p}_{un}")
                        q.dma_start(out=w2t[:], in_=w2f8_d[p, un, :, :, :])
                        for jj in range(4):
                            nc.tensor.matmul(
                                psY[jj][:],
                                lhsT=u8p[:, un, :, P * jj:P * (jj + 1)],
                                rhs=w2t[:],
                                perf_mode=mybir.MatmulPerfMode.DoubleRow,
                                start=(un == 0), stop=(un == nunit - 1))
                    else:
                        w2t = pool.tile([P, 512], BF16, tag=tg,
                                        name=f"w2t_{p}_{un}")
                        q.dma_start(out=w2t[:],
                                    in_=w2bf_d[p, un - N2P, :, :])
                        for jj in range(4):
                            nc.tensor.matmul(
                                psY[jj][:],
                                lhsT=ubf[:, un - N2P, P * jj:P * (jj + 1)],
                                rhs=w2t[:],
                                start=(un == 0), stop=(un == nunit - 1))
                for jj in range(4):
                    # out = (psY + 16*x2_true) / 16: add in psum, scaled copy
                    nc.vector.tensor_tensor(
                        psY[jj][:], psY[jj][:], x2t[jj][:, 512 * p:512 * (p + 1)],
                        mybir.AluOpType.add)
                    ot = p_x.tile([P, 512], BF16, tag="xt", name=f"ot_{p}_{jj}", bufs=3)
                    nc.scalar.activation(ot[:], psY[jj][:],
                                         mybir.ActivationFunctionType.Copy,
                                         scale=1.0 / X2S)
                    nc.scalar.dma_start(
                        out=out_d[P * jj:P * (jj + 1), 512 * p:512 * (p + 1)],
                        in_=ot[:])

            # ================= emission schedule ==============================
            for t in range(2):
                xpre[t] = emit_x_tile(t)
            emit_weight_dmas()
            for g in range(NG_PER_B):
                for tt in range(2):
                    tn = 2 * (g + 1) + tt
                    if tn < 16:
                        xpre[tn] = emit_x_tile(tn)
                emit_A_group(0, g)

            # attention(b0) interleaved with QKV(b1)
            for qt in range(QT_PER_B):
                if qt % 2 == 0:
                    for tt in range(2):
                        xpre[16 + qt + tt] = emit_x_tile(16 + qt + tt)
                else:
                    emit_A_group(1, qt // 2)
                exs = [emit_B_S(0, qt, lh) for lh in range(HL)]
                for lh in range(HL):
                    emit_B_AV(0, qt, lh, exs[lh])
                if qt == 1:
                    nc.scalar.dma_start(out=wo_sb[:], in_=wo_d[:, :, :])
                if qt == 3:
                    for j in range(NCHUNK):
                        emit_xres(j)
            emit_collective(0)

            # attention(b1) interleaved with chunk 0/1 post-processing
            for qt in range(QT_PER_B - 1, -1, -1):
                exs = [emit_B_S(1, qt, lh) for lh in range(HL)]
                for lh in range(HL):
                    emit_B_AV(1, qt, lh, exs[lh])
                if qt == 7:
                    emit_C1(0)
                if qt == 6:
                    emit_C2(0)
                if qt == 5:
                    emit_C1(1)
                if qt == 4:
                    emit_C2(1)
            emit_collective(1)

            # early MLP1: fp8 tiles, first token half (chunks 0,1) -- fills the
            # AllToAll latency window; their W1 tiles are streamed again for
            # the second half (cheap: 5MB fp8)
            for mm in range(N1F):
                emit_D_tile(mm, 0, 256)
            emit_C1(2)
            emit_C2(2)
            emit_C1(3)
            emit_C2(3)
            # now the kq region is dead (attention complete) -> bf16 u tiles
            ubf = p_kvu.tile([P, MMT - N2F, 512], BF16, tag="kvu", name="ubf")
            for mm in range(N1F):
                emit_D_tile(mm, 256, 256)
            for mm in range(N1F, MMT):
                emit_D_tile(mm, 0, 512)
            for p in range(NPASS):
                emit_E_pass(p)
    nc.compile()
    return nc


def _bf16(a):
    return np.asarray(a, dtype=np.float32).astype(ml_dtypes.bfloat16)


def _fp8(a):
    return np.clip(np.asarray(a, np.float32), -240, 240).astype(mybir.dt.np(FP8))


def make_in_maps(x, Wq, Wk, Wv, Wo, g1, bn1, g2, bn2, W1, b1, W2, b2):
    x = np.asarray(x, np.float32)
    x_flat = np.ascontiguousarray(x.reshape(NTOK, H))

    wq_eff = (g1[:, None] * np.asarray(Wq, np.float32)) * WSCALE
    wk_eff = (g1[:, None] * np.asarray(Wk, np.float32)) * WSCALE
    wv_eff = (g1[:, None] * np.asarray(Wv, np.float32)) * WSCALE
    bq = (bn1 @ np.asarray(Wq, np.float32)) * WSCALE
    bk = (bn1 @ np.asarray(Wk, np.float32)) * WSCALE
    bv = (bn1 @ np.asarray(Wv, np.float32)) * WSCALE
    w1_eff = g2[:, None] * np.asarray(W1, np.float32)
    b1_eff = np.asarray(b1, np.float32) + bn2 @ np.asarray(W1, np.float32)

    xbf = np.ascontiguousarray(_bf16(x_flat))
    # W1: [mm, p, ks, mw]; tiles 0..N1F-1 fp8 (x16), rest bf16
    w1_t = _bf16(w1_eff).astype(np.float32).reshape(KS, P, MMT, P).transpose(2, 1, 0, 3)
    w1f8 = np.ascontiguousarray(_fp8(16.0 * w1_eff.reshape(KS, P, MMT, P)
                                     .transpose(2, 1, 0, 3)[:N1F]))
    w1bf = np.ascontiguousarray(_bf16(w1_eff.reshape(KS, P, MMT, P)
                                      .transpose(2, 1, 0, 3)[N1F:]))
    # W2 scaled by 32 on both dtypes; [pass][unit][...]
    W2f = np.asarray(W2, np.float32) * 16.0
    w2f8 = np.empty((NPASS, N2P, P, 2, 512), mybir.dt.np(FP8))
    w2bf = np.empty((NPASS, NBF2, P, 512), ml_dtypes.bfloat16)
    for p in range(NPASS):
        cols = slice(512 * p, 512 * (p + 1))
        for q in range(N2P):
            w2f8[p, q, :, 0, :] = _fp8(W2f[P * 2 * q:P * (2 * q + 1), cols])
            w2f8[p, q, :, 1, :] = _fp8(W2f[P * (2 * q + 1):P * (2 * q + 2), cols])
        for i in range(NBF2):
            mm = N2F + i
            w2bf[p, i] = _bf16(W2f[P * mm:P * (mm + 1), cols])
    b1m = np.ascontiguousarray(b1_eff.reshape(MMT, P).T.astype(np.float32))
    wo8 = np.ascontiguousarray(
        _fp8(16.0 * np.asarray(Wo, np.float32)).reshape(KS, P, H).transpose(1, 0, 2))
    ii, jj_ = np.meshgrid(np.arange(P), np.arange(P), indexing="ij")
    cmaskT = np.where(ii <= jj_, 0.0, NEG).astype(np.float32)
    b2f = np.asarray(b2, np.float32)

    in_maps = []
    for c in range(NCORES):
        cs = slice(DV * c, DV * (c + 1))
        wqk = np.concatenate([wq_eff[:, cs], wk_eff[:, cs]], axis=1)  # [H, 512]
        wqk_t = np.ascontiguousarray(
            _fp8(wqk).reshape(KS, P, DQK).transpose(1, 0, 2))
        bqk = np.concatenate([bq[cs], bk[cs]]).astype(np.float32)
        bqk_m = np.ascontiguousarray(bqk.reshape(DQK // P, P).T)
        wv_t = np.ascontiguousarray(
            _fp8(wv_eff[:, cs]).reshape(KS, P, DV).transpose(1, 0, 2))
        bvbc = np.ascontiguousarray(np.broadcast_to(
            bv[cs].astype(np.float32).reshape(1, HL, P), (P, HL, P)))
        xres = np.concatenate(
            [x_flat[1024 * j + P * c:1024 * j + P * (c + 1)] for j in range(NCHUNK)],
            axis=0) + b2f
        xres16 = np.ascontiguousarray((16.0 * xres).astype(np.float32))
        in_maps.append({
            "xbf": xbf, "xres": xres16,
            "wqk": wqk_t, "bqk": bqk_m, "wv": wv_t, "bvbc": bvbc, "wo": wo8,
            "w1f8": w1f8, "w1bf": w1bf, "b1": b1m, "w2f8": w2f8, "w2bf": w2bf,
            "cmaskT": cmaskT,
        })
    return in_maps


_NC_CACHE = {}


def kernel(**inputs):
    if "nc" not in _NC_CACHE:
        _NC_CACHE["nc"] = build()
    nc = _NC_CACHE["nc"]
    in_maps = make_in_maps(
        inputs["x"], inputs["Wq"], inputs["Wk"], inputs["Wv"], inputs["Wo"],
        np.asarray(inputs["g1"], np.float32), np.asarray(inputs["bn1"], np.float32),
        np.asarray(inputs["g2"], np.float32), np.asarray(inputs["bn2"], np.float32),
        inputs["W1"], inputs["b1"], inputs["W2"], inputs["b2"])
    res = run_bass_kernel_spmd(nc, in_maps, list(range(NCORES)))
    out = np.empty((NTOK, H), np.float32)
    for c in range(NCORES):
        oc = np.asarray(res.results[c]["out"], dtype=np.float32)
        for j in range(NCHUNK):
            out[1024 * j + P * c:1024 * j + P * (c + 1)] = oc[P * j:P * (j + 1)]
    return out.reshape(B, T, H)


# revision 20
# speedup vs baseline: 1.2625x; 1.0330x over previous
"""Fused transformer block (LN -> causal MHA -> residual -> LN -> SiLU MLP -> residual)
on 8 Trainium2 NeuronCores.

v3 design (on top of the v2 baseline):
- Tensor-parallel over heads (2 heads/core) for QKV + attention; S computed
  transposed (S^T) with the softmax denominator as a ones-column of V.
- Attention outputs are TRANSPOSED and quantized to fp8 BEFORE the AllToAll
  (payload halves to 512KB/group; after the exchange the received buffer is
  directly the lhsT of the O-projection -> no post-collective PE work beyond
  the matmuls themselves).
- x2 (attention residual) is kept in SBUF in f32, scaled by 32 so that the
  fp8(32*W2) and bf16(32*W2) MLP2 products accumulate uniformly; the final
  drain rescales by 1/32. LN2 is scale-invariant (eps scaled to match).
- MLP is FUSED: u = silu(mlp1) stays in SBUF (aliased onto the dead K/Q
  SBUF region); no DRAM round trip. MLP2 runs in 4 feature passes of 4 PSUM
  banks each.
- Mixed-precision MLP: mid-tiles 0..19 of MLP1 and mid-tile pairs 0..10 of
  MLP2 run fp8+DoubleRow (1.8x); the rest bf16. Chosen so the predicted
  rel-err (numpy-emulated, matches HW to 3 digits) is ~1.8e-2 < 2e-2.
- The first 20 MLP1 tiles run as two 256-token halves so their first halves
  (token chunks 0,1, available right after C2(1)) fill the ~50us AllToAll #2
  latency window; their fp8 W1 tiles are streamed twice (5MB extra).
- PSUM ring decoupling: attention (ring qs/sm), C (own ring c), so the
  collective-dependent O-projection never blocks attention PSUM reuse.
- Output written bf16 (host upcasts), xres folded with b2 and pre-scaled on
  the host.
"""

import sys
import os

for _p in ("/opt/trn_rl_repo", "/root/.axon_site/_ro/trn_rl_repo"):
    if os.path.isdir(_p) and _p not in sys.path:
        sys.path.insert(0, _p)
        break

import numpy as np
import ml_dtypes

import concourse.bass as bass
from concourse import bacc
import concourse.mybir as mybir
import concourse.tile as tile
from concourse.masks import make_identity
from concourse.bass_utils import run_bass_kernel_spmd

F32 = mybir.dt.float32
BF16 = mybir.dt.bfloat16
FP8 = mybir.dt.float8e4


def _install_act_table_hint():
    """Steer the act-table-set chooser so Exp and Ln resolve to the one set
    that contains BOTH (natural_log_exp_and_others)."""
    import concourse.bacc as _bacc
    if getattr(_bacc, "_act_hint_installed", False):
        return
    _orig = _bacc.get_activation_tables

    def _patched(arch):
        tabs = _orig(arch)
        exp = mybir.ActivationFunctionType.Exp
        ln = mybir.ActivationFunctionType.Ln
        for name, fns in tabs.items():
            if name != "natural_log_exp_and_others":
                fns.discard(exp)
                fns.discard(ln)
        return tabs

    _bacc.get_activation_tables = _patched
    _bacc._act_hint_installed = True


_install_act_table_hint()

P = 128          # partitions / head_dim / token tile
H = 2048         # hidden
KS = H // P      # 16 k-subtiles over hidden
HEADS = 16
HL = 2           # heads per core
NCORES = 8
B = 2
T = 2048
NTOK = B * T     # 4096
TPB = T          # tokens per batch
MID = 4 * H      # 8192
MMT = MID // P   # 64 m-tiles over mid dim
DQK = 2 * HL * P   # 512 rows of fused QK projection per core
DV = HL * P        # 256 V/attention-out features per core
EPS = 1e-5
NEG = -1.0e30

QT_PER_B = TPB // P   # 16 q tiles per batch
MT = NTOK // P        # 32 token m-tiles
NCHUNK = 4            # token chunks per core (128 each)
GT = 256              # tokens per A-group
WSCALE = 16.0         # fp8 weight rescale (avoids e4m3 subnormals)
SCORE_SCALE = (1.0 / np.sqrt(P)) / (WSCALE * WSCALE)
NG_PER_B = TPB // GT  # 8 A-groups per batch

# ---- mixed-precision MLP config ----
N1F = 20              # MLP1 mid-tiles 0..N1F-1 in fp8 DoubleRow (also the
                      # "early" tiles run as two 256-token halves)
N2P = 11              # MLP2 mid-tile PAIRS 0..N2P-1 (tiles 0..21) in fp8 DR
N2F = 2 * N2P         # fp8 MLP2 tiles
NBF2 = MMT - N2F      # 42 bf16 MLP2 tiles
NPASS = 4             # MLP2 feature passes (512 cols each)
X2S = 16.0            # x2 kept as 16*x2_true in SBUF


def build(sim=False, trn_kwargs=None, trace_sim=False):
    nc = bacc.Bacc(None, num_devices=NCORES, **(trn_kwargs or {}))

    x_d = nc.declare_dram_parameter("xbf", [NTOK, H], FP8, isOutput=False)
    xres_d = nc.declare_dram_parameter("xres", [NCHUNK * P, H], F32, isOutput=False)
    wqk_d = nc.declare_dram_parameter("wqk", [P, KS, DQK], FP8, isOutput=False)
    bqk_d = nc.declare_dram_parameter("bqk", [P, DQK // P], F32, isOutput=False)
    wv_d = nc.declare_dram_parameter("wv", [P, KS, DV], FP8, isOutput=False)
    bvbc_d = nc.declare_dram_parameter("bvbc", [P, HL, P], F32, isOutput=False)
    wo_d = nc.declare_dram_parameter("wo", [P, KS, H], FP8, isOutput=False)
    w1f8_d = nc.declare_dram_parameter("w1f8", [N1F, P, KS, P], FP8, isOutput=False)
    w1bf_d = nc.declare_dram_parameter("w1bf", [MMT - N1F, P, KS, P], BF16,
                                       isOutput=False)
    b1_d = nc.declare_dram_parameter("b1", [P, MMT], F32, isOutput=False)
    w2f8_d = nc.declare_dram_parameter("w2f8", [NPASS, N2P, P, 2, 512], FP8,
                                       isOutput=False)
    w2bf_d = nc.declare_dram_parameter("w2bf", [NPASS, NBF2, P, 512], BF16,
                                       isOutput=False)
    cmaskT_d = nc.declare_dram_parameter("cmaskT", [P, P], F32, isOutput=False)
    out_d = nc.declare_dram_parameter("out", [NCHUNK * P, H], BF16, isOutput=True)

    from contextlib import ExitStack
    with tile.TileContext(nc, trace_sim=trace_sim) as tc:
        with ExitStack() as stack:
            dram = stack.enter_context(tc.tile_pool(name="dram", bufs=1, space="DRAM"))
            const = stack.enter_context(tc.tile_pool(name="const", bufs=1))
            wbig = stack.enter_context(tc.tile_pool(name="wbig", bufs=1))
            # wqk (8KB/part, dead after QKV) chained with h2T (16KB)
            p_ali = stack.enter_context(tc.tile_pool(name="ali16", bufs=1))
            # ksb+qT (32KB, dead after last AV) chained with ubf (42KB)
            p_kvu = stack.enter_context(tc.tile_pool(name="kvu", bufs=1))
            p_vsb = stack.enter_context(tc.tile_pool(name="vsb", bufs=2))
            p_u8 = stack.enter_context(tc.tile_pool(name="u8", bufs=1))
            p_x = stack.enter_context(tc.tile_pool(name="xin", bufs=2))
            p_ln = stack.enter_context(tc.tile_pool(name="lnsmall", bufs=2))
            p_h = stack.enter_context(tc.tile_pool(name="htok", bufs=2))
            p_hT = stack.enter_context(tc.tile_pool(name="hT", bufs=2))
            p_h2T8 = stack.enter_context(tc.tile_pool(name="h2T8", bufs=1))
            p_ex = stack.enter_context(tc.tile_pool(name="expT", bufs=2))
            p_ao = stack.enter_context(tc.tile_pool(name="aot", bufs=2))
            p_aoT = stack.enter_context(tc.tile_pool(name="aoT", bufs=1))
            p_afT = stack.enter_context(tc.tile_pool(name="afT", bufs=1))
            p_x2 = stack.enter_context(tc.tile_pool(name="x2", bufs=4))
            p_w1 = stack.enter_context(tc.tile_pool(name="w1pool", bufs=2))
            p_w2 = stack.enter_context(tc.tile_pool(name="w2pool", bufs=2))
            # PSUM rings:
            #  qs: 4 x 2KB  (A-QK, A-transposes, B-S, D-full psU, E-psY)
            #  sm: 4 x 1KB  (A-V, B-AV, B-aoT transposes, D-early psU halves)
            #  c : 2 x 2KB  (C O-proj, C2 h2 transposes) -- collective-coupled
            ps_qs = stack.enter_context(tc.tile_pool(name="psqs", bufs=4, space="PSUM"))
            ps_sm = stack.enter_context(tc.tile_pool(name="pssm", bufs=2, space="PSUM"))
            ps_c = stack.enter_context(tc.tile_pool(name="psc", bufs=2, space="PSUM"))

            # ---- internal DRAM ----
            # aotT laid [g][dst s][jj][fsub][f][t]; per-(g,s) shard contiguous
            aot_dram = dram.tile([2, NCORES * 2 * 2 * P, P], BF16)
            a2a_dram = dram.tile([2, NCORES * 2 * 2 * P, P], BF16)

            # ---- constants / weights in SBUF ----
            ident = const.tile([P, P], BF16)
            make_identity(nc, ident)
            epsb = const.tile([P, 1], F32)
            nc.vector.memset(epsb[:], EPS)
            epsb2 = const.tile([P, 1], F32)
            nc.vector.memset(epsb2[:], EPS * X2S * X2S)
            scrap = const.tile([P, 1], F32)
            cmaskT = const.tile([P, P], F32)
            nc.sync.dma_start(cmaskT[:], cmaskT_d[:, :])
            bqk_sb = const.tile([P, DQK // P], F32)
            nc.sync.dma_start(bqk_sb[:], bqk_d[:, :])
            bvbc_sb = const.tile([P, HL, P], F32)
            nc.sync.dma_start(bvbc_sb[:], bvbc_d[:, :, :])
            b1_sb = const.tile([P, MMT], F32)
            nc.sync.dma_start(b1_sb[:], b1_d[:, :])
            wqk_sb = p_ali.tile([P, KS, DQK], FP8, tag="ali16", name="wqk_sb")
            wv_sb = wbig.tile([P, KS, DV], FP8)
            wo_sb = wbig.tile([P, KS, H], FP8)

            def emit_weight_dmas():
                nc.gpsimd.dma_start(out=wqk_sb[:, :KS // 2, :],
                                    in_=wqk_d[:, :KS // 2, :])
                nc.scalar.dma_start(out=wqk_sb[:, KS // 2:, :],
                                    in_=wqk_d[:, KS // 2:, :])
                nc.gpsimd.dma_start(out=wv_sb[:], in_=wv_d[:, :, :])

            def layer_norm_stats(parts, name, tagp="", eps=None):
                """parts: list of (tile, ncols512). Returns (nmu, rstd) [P,1]."""
                st = p_ln.tile([P, 4, 6], F32, tag=tagp + "lnst", name=f"st_{name}")
                a = 0
                for tile_, n in parts:
                    for i in range(n):
                        nc.vector.bn_stats(st[:, a, :], tile_[:, 512 * i:512 * (i + 1)])
                        a += 1
                assert a == 4
                mv = p_ln.tile([P, 2], F32, tag=tagp + "lnmv", name=f"mv_{name}")
                nc.vector.bn_aggr(mv[:], st[:])
                lv = p_ln.tile([P, 1], F32, tag=tagp + "lnsd", name=f"lv_{name}")
                nc.scalar.activation(lv[:], mv[:, 1:2],
                                     mybir.ActivationFunctionType.Ln,
                                     bias=(eps if eps is not None else epsb)[:])
                rstd = p_ln.tile([P, 1], F32, tag=tagp + "lnrstd", name=f"rstd_{name}")
                nc.scalar.activation(rstd[:], lv[:],
                                     mybir.ActivationFunctionType.Exp, scale=-0.5)
                nmu = p_ln.tile([P, 1], F32, tag=tagp + "lnnmu", name=f"nmu_{name}")
                nc.vector.tensor_scalar_mul(nmu[:], mv[:, 0:1], -1.0)
                return nmu[:], rstd[:]

            def ln_apply(dst, src, nmu, rstd, engine):
                engine.tensor_scalar(dst, src, nmu, rstd,
                                     mybir.AluOpType.add, mybir.AluOpType.mult)

            # ================= Stage A: LN1, transpose, QKV ===================
            # ksb/qT live in one big tile so the whole region can be reused by
            # the bf16 u tiles of the fused MLP once attention is done.
            kq_all = p_kvu.tile([P, 2, 2, HL, TPB], BF16, tag="kvu", name="kq_all")

            def ksb(b):
                return kq_all[:, b, 0]

            def qT(b):
                return kq_all[:, b, 1]

            vsb = [None, None]
            xpre = {}

            def emit_x_tile(t):
                xh = []
                for hh in range(2):
                    xth = p_x.tile([P, H // 2], FP8, tag="xt",
                                   name=f"xt_{t}_{hh}", bufs=6)
                    (nc.sync if hh == 0 else nc.scalar).dma_start(
                        out=xth[:], in_=x_d[P * t:P * (t + 1),
                                           (H // 2) * hh:(H // 2) * (hh + 1)])
                    xh.append(xth)
                return xh

            def emit_A_group(b, g):
                """LN1 + transpose + QKV for GT=256 tokens (group g of batch b)."""
                if g == 0:
                    vsb[b] = p_vsb.tile([P, QT_PER_B, HL, P + 2], BF16, tag="vsb",
                                        name=f"vsb_{b}")
                    nc.vector.memset(vsb[b][:, :, :, P:P + 1], 1.0)
                hT = p_hT.tile([P, KS, GT], FP8, tag="hT", name=f"hT_{b}_{g}")
                if b == 0:
                    ev_copy = lambda out, in_: nc.scalar.copy(out=out, in_=in_)
                    ev_bias = lambda out, in_, s: nc.scalar.add(out, in_, s)
                else:
                    ev_copy = lambda out, in_: nc.vector.tensor_copy(out=out, in_=in_)
                    ev_bias = lambda out, in_, s: nc.vector.tensor_scalar_add(
                        out, in_, s)
                for tt in range(GT // P):   # 128-token LN tiles
                    t = (TPB * b + GT * g) // P + tt
                    xh = xpre.pop(t, None)
                    if xh is None:
                        xh = emit_x_tile(t)
                    nmu, rstd = layer_norm_stats([(xh[0], 2), (xh[1], 2)],
                                                 f"ln1_{t}")
                    ht = p_h.tile([P, H], BF16, tag="ht", name=f"ht_{t}", bufs=2)
                    for hh in range(2):
                        ln_apply(ht[:, (H // 2) * hh:(H // 2) * (hh + 1)],
                                 xh[hh][:], nmu, rstd, nc.gpsimd)
                    for fg in range(KS // 8):
                        ptp = ps_c.tile([P, 1024], BF16, tag="psc",
                                        name=f"trp_{t}_{fg}")
                        for f4 in range(8):
                            f = 8 * fg + f4
                            nc.tensor.transpose(ptp[:, P * f4:P * (f4 + 1)],
                                                ht[:, P * f:P * (f + 1)], ident[:])
                        ev_copy(hT[:, 8 * fg:8 * (fg + 1), P * tt:P * (tt + 1)],
                                ptp[:].rearrange("p (a b) -> p a b", b=P))

                col0 = GT * g
                # QK projection: m 0,1 -> Q head0/1 ; 2,3 -> K head0/1
                for m in range(4):
                    ps = ps_qs.tile([P, GT], F32, tag="psqs", name=f"qk_{b}_{g}_{m}")
                    for k2 in range(KS // 2):
                        nc.tensor.matmul(
                            ps[:], lhsT=wqk_sb[:, 2 * k2:2 * k2 + 2, P * m:P * (m + 1)],
                            rhs=hT[:, 2 * k2:2 * k2 + 2, :],
                            perf_mode=mybir.MatmulPerfMode.DoubleRow,
                            start=(k2 == 0), stop=(k2 == KS // 2 - 1))
                    dst = qT(b) if m < 2 else ksb(b)
                    ev_bias(dst[:, m % 2, col0:col0 + GT], ps[:],
                            bqk_sb[:, m:m + 1])
                # V projection (token-major)
                for m in range(GT // P):
                    ps = ps_sm.tile([P, DV], F32, tag="pssm", name=f"v_{b}_{g}_{m}")
                    for k2 in range(KS // 2):
                        nc.tensor.matmul(
                            ps[:], lhsT=hT[:, 2 * k2:2 * k2 + 2, P * m:P * (m + 1)],
                            rhs=wv_sb[:, 2 * k2:2 * k2 + 2, :],
                            perf_mode=mybir.MatmulPerfMode.DoubleRow,
                            start=(k2 == 0), stop=(k2 == KS // 2 - 1))
                    tm = (GT * g) // P + m
                    nc.vector.tensor_tensor(
                        vsb[b][:, tm, :, 0:P],
                        ps[:].rearrange("p (a b) -> p a b", b=P),
                        bvbc_sb[:], mybir.AluOpType.add)

            # ================= Stage B: attention (S^T form) ==================
            aosb = {}

            def emit_B_S(b, qt, lh):
                """S^T matmuls + mask + exp for (batch, query tile, local head)."""
                klen = P * (qt + 1)
                nchs = (qt + 4) // 4
                ex = p_ex.tile([P, TPB], BF16, tag="ex", name=f"ex_{b}_{qt}_{lh}")
                qcols = qT(b)[:, lh, P * qt:P * (qt + 1)]
                for j in range(nchs):
                    n0 = 512 * j
                    n1 = min(n0 + 512, klen)
                    ps = ps_qs.tile([P, 512], F32, tag="psqs",
                                    name=f"s_{b}_{qt}_{lh}_{j}")
                    for kb in range(n0 // P, n1 // P):
                        nc.tensor.matmul(ps[:, P * kb - n0:P * (kb + 1) - n0],
                                         lhsT=ksb(b)[:, lh, P * kb:P * (kb + 1)],
                                         rhs=qcols, start=True, stop=True)
                    if j == nchs - 1:
                        d0 = klen - P - n0
                        nc.vector.tensor_tensor(ps[:, d0:d0 + P], ps[:, d0:d0 + P],
                                                cmaskT[:], mybir.AluOpType.add)
                    nc.scalar.activation(ex[:, n0:n1], ps[:, :n1 - n0],
                                         mybir.ActivationFunctionType.Exp,
                                         scale=float(SCORE_SCALE))
                return ex

            def emit_B_AV(b, qt, lh, ex):
                """A@V with ones-column, normalize; transpose + fp8-stage after
                lh=1 so the a2a payload is already in O-projection lhsT form."""
                mt = QT_PER_B * b + qt
                if lh == 0:
                    aosb[mt] = p_ao.tile([P, HL, P], BF16, tag="aot", name=f"ao_{mt}")
                psO = ps_sm.tile([P, P + 2], F32, tag="pssm", name=f"o_{mt}_{lh}")
                for kb in range(qt + 1):
                    nc.tensor.matmul(psO[:, :P + 1],
                                     lhsT=ex[:, P * kb:P * (kb + 1)],
                                     rhs=vsb[b][:, kb, lh, 0:P + 1],
                                     start=(kb == 0), stop=(kb == qt))
                rinv = p_ln.tile([P, 1], F32, tag="rinv", name=f"ri_{mt}_{lh}")
                nc.vector.reciprocal(rinv[:], psO[:, P:P + 1])
                # aosb = attn_true (v carries the 16x weight scale; /16 here)
                nc.vector.tensor_scalar(aosb[mt][:, lh, :], psO[:, 0:P],
                                        rinv[:], 1.0 / 16.0,
                                        mybir.AluOpType.mult, mybir.AluOpType.mult)
                if lh == HL - 1:
                    # transpose [tok, 2*128f] -> [2, 128f, tok], cast fp8, stage
                    ptp = ps_sm.tile([P, HL * P], BF16, tag="pssm",
                                     name=f"aop_{mt}")
                    for hh in range(HL):
                        nc.tensor.transpose(ptp[:, P * hh:P * (hh + 1)],
                                            aosb[mt][:, hh, :], ident[:])
                    aoT = p_aoT.tile([P, HL, P], BF16, tag="aoT", name=f"aoT_{mt}")
                    nc.scalar.copy(
                        out=aoT[:],
                        in_=ptp[:].rearrange("p (a b) -> p a b", b=P))
                    g, s, jj = mt // 16, mt % 8, (mt // 8) % 2
                    r0 = 512 * s + 256 * jj
                    nc.sync.dma_start(
                        aot_dram[g, r0:r0 + 256, :].rearrange(
                            "(a p) t -> p a t", a=HL),
                        aoT[:])
                    del aosb[mt]

            rg = [list(range(NCORES))]

            def emit_collective(g):
                nc.gpsimd.collective_compute(
                    "AllToAll", mybir.AluOpType.bypass, replica_groups=rg,
                    ins=[aot_dram[g, :, :]], outs=[a2a_dram[g, :, :]])

            # ================= Stage C: O-proj + LN2 per chunk ================
            h2T = p_ali.tile([P, KS, NCHUNK * P], BF16, tag="ali16", name="h2T")
            h2T8 = p_h2T8.tile([P, KS, NCHUNK * P], FP8, tag="h2T8", name="h2T8")
            x2t = [None] * NCHUNK
            c_state = {}

            def emit_xres(j):
                """xres (host: 16*(x+b2), f32) lands directly in the x2 tile."""
                x2t[j] = p_x2.tile([P, H], F32, tag="x2keep", name=f"x2_{j}")
                nc.scalar.dma_start(x2t[j][:], xres_d[P * j:P * (j + 1), :])

            def emit_C1(j):
                """a2a readback (already fp8 lhsT) + O-proj + scaled residual."""
                g, jj = j // 2, j % 2
                afT = p_afT.tile([P, KS, P], FP8, tag="afT", name=f"afT_{j}")
                a2av = a2a_dram[g].rearrange("(s j f p) t -> p s j f t",
                                             s=NCORES, j=2, p=P)
                for fs in range(2):
                    nc.gpsimd.dma_start(
                        out=afT[:].rearrange("p (s f) t -> p s f t",
                                             s=NCORES)[:, :, fs],
                        in_=a2av[:, :, jj, fs])
                x2 = x2t[j]
                for nn in range(4):
                    psn = ps_c.tile([P, 512], F32, tag="psc", name=f"op_{j}_{nn}")
                    for k2 in range(KS // 2):
                        nc.tensor.matmul(
                            psn[:], lhsT=afT[:, 2 * k2:2 * k2 + 2, :],
                            rhs=wo_sb[:, 2 * k2:2 * k2 + 2, 512 * nn:512 * (nn + 1)],
                            perf_mode=mybir.MatmulPerfMode.DoubleRow,
                            start=(k2 == 0), stop=(k2 == KS // 2 - 1))
                    c0 = 512 * nn
                    # x2 = psO2 + 16*(x+b2)  -> 16 * x2_true   (in place)
                    nc.vector.tensor_tensor(
                        x2[:, c0:c0 + 512], psn[:], x2[:, c0:c0 + 512],
                        mybir.AluOpType.add)
                nmu, rstd = layer_norm_stats([(x2, 4)], f"ln2_{j}", tagp="c2",
                                             eps=epsb2)
                c_state[j] = (x2, nmu, rstd)

            def emit_C2(j):
                """LN2 apply + h2 transpose into h2T (bf16) and h2T8 (fp8)."""
                x2, nmu, rstd = c_state.pop(j)
                h2 = p_h.tile([P, H], BF16, tag="ht", name=f"h2_{j}", bufs=2)
                for hh in range(2):
                    ln_apply(h2[:, (H // 2) * hh:(H // 2) * (hh + 1)],
                             x2[:, (H // 2) * hh:(H // 2) * (hh + 1)],
                             nmu, rstd, nc.gpsimd)
                for fg in range(KS // 8):
                    ptp = ps_c.tile([P, 1024], BF16, tag="psc", name=f"h2t_{j}_{fg}")
                    for f4 in range(8):
                        f = 8 * fg + f4
                        nc.tensor.transpose(ptp[:, P * f4:P * (f4 + 1)],
                                            h2[:, P * f:P * (f + 1)], ident[:])
                    nc.vector.tensor_copy(
                        out=h2T[:, 8 * fg:8 * (fg + 1), P * j:P * (j + 1)],
                        in_=ptp[:].rearrange("p (a b) -> p a b", b=P))
                    nc.scalar.copy(
                        out=h2T8[:, 8 * fg:8 * (fg + 1), P * j:P * (j + 1)],
                        in_=ptp[:].rearrange("p (a b) -> p a b", b=P))

            # ================= Stage D: MLP1 (fused, u stays in SBUF) =========
            silu_fn = (mybir.ActivationFunctionType.Sigmoid if sim
                       else mybir.ActivationFunctionType.Silu)
            # u8p: fp8 mid-tiles 0..N2F-1 as DoubleRow pairs; ubf: tiles N2F..63
            u8p = p_u8.tile([P, N2P, 2, 512], FP8, tag="u8p", name="u8p")
            ubf = None   # allocated after attention frees kq_all

            def u_dst(mm, c0, cn):
                if mm < N2F:
                    return u8p[:, mm // 2, mm % 2, c0:c0 + cn]
                return ubf[:, mm - N2F, c0:c0 + cn]

            def emit_D_tile(mm, c0, cn, w1t=None):
                """MLP1 mid-tile mm over token cols [c0, c0+cn)."""
                fp8 = mm < N1F
                if w1t is None:
                    pool, tg = ((p_w1, "w1t") if mm % 2 == 0 else (p_hT, "hT"))
                    q = nc.gpsimd if mm % 2 == 0 else nc.sync
                    if fp8:
                        w1t = pool.tile([P, KS, P], FP8, tag=tg,
                                        name=f"w1t_{mm}_{c0}")
                        q.dma_start(out=w1t[:], in_=w1f8_d[mm, :, :, :])
                    else:
                        w1t = pool.tile([P, KS, P], BF16, tag=tg,
                                        name=f"w1t_{mm}_{c0}")
                        q.dma_start(out=w1t[:], in_=w1bf_d[mm - N1F, :, :, :])
                if cn == 512:
                    ps = ps_qs.tile([P, 512], F32, tag="psqs", name=f"u_{mm}")
                else:
                    ps = ps_sm.tile([P, cn], F32, tag="pssm", name=f"u_{mm}_{c0}")
                if fp8:
                    for k2 in range(KS // 2):
                        nc.tensor.matmul(
                            ps[:], lhsT=w1t[:, 2 * k2:2 * k2 + 2, :],
                            rhs=h2T8[:, 2 * k2:2 * k2 + 2, c0:c0 + cn],
                            perf_mode=mybir.MatmulPerfMode.DoubleRow,
                            start=(k2 == 0), stop=(k2 == KS // 2 - 1))
                    sc = 1.0 / 16.0
                else:
                    for ks in range(KS):
                        nc.tensor.matmul(ps[:], lhsT=w1t[:, ks, :],
                                         rhs=h2T[:, ks, c0:c0 + cn],
                                         start=(ks == 0), stop=(ks == KS - 1))
                    sc = 1.0
                nc.scalar.activation(u_dst(mm, c0, cn), ps[:], silu_fn,
                                     bias=b1_sb[:, mm:mm + 1], scale=sc)
                return w1t

            # ================= Stage E: MLP2 (4 feature passes) ===============
            def emit_E_pass(p):
                psY = [ps_qs.tile([P, 512], F32, tag="psqs", name=f"y_{p}_{jj}")
                       for jj in range(4)]
                nunit = N2P + NBF2
                for un in range(nunit):
                    pool, tg = ((p_w2, "w2t") if un % 2 == 0 else (p_vsb, "vsb"))
                    q = nc.sync if un % 2 == 0 else nc.scalar
                    if un < N2P:
                        w2t = pool.tile([P, 2, 512], FP8, tag=tg,
                                        name=f"w2t_{p}_{un}")
                        q.dma_start(out=w2t[:], in_=w2f8_d[p, un, :, :, :])
                        for jj in range(4):
                            nc.tensor.matmul(
                                psY[jj][:],
                                lhsT=u8p[:, un, :, P * jj:P * (jj + 1)],
                                rhs=w2t[:],
                                perf_mode=mybir.MatmulPerfMode.DoubleRow,
                                start=(un == 0), stop=(un == nunit - 1))
                    else:
                        w2t = pool.tile([P, 512], BF16, tag=tg,
                                        name=f"w2t_{p}_{un}")
                        q.dma_start(out=w2t[:],
                                    in_=w2bf_d[p, un - N2P, :, :])
                        for jj in range(4):
                            nc.tensor.matmul(
                                psY[jj][:],
                                lhsT=ubf[:, un - N2P, P * jj:P * (jj + 1)],
                                rhs=w2t[:],
                                start=(un == 0), stop=(un == nunit - 1))
                for jj in range(4):
                    # out = (psY + 16*x2_true) / 16: add in psum, scaled copy
                    nc.vector.tensor_tensor(
                        psY[jj][:], psY[jj][:], x2t[jj][:, 512 * p:512 * (p + 1)],
                        mybir.AluOpType.add)
                    ot = p_x.tile([P, 512], BF16, tag="xt", name=f"ot_{p}_{jj}", bufs=6)
                    nc.scalar.activation(ot[:], psY[jj][:],
                                         mybir.ActivationFunctionType.Copy,
                                         scale=1.0 / X2S)
                    nc.scalar.dma_start(
                        out=out_d[P * jj:P * (jj + 1), 512 * p:512 * (p + 1)],
                        in_=ot[:])

            # ================= emission schedule ==============================
            for t in range(2):
                xpre[t] = emit_x_tile(t)
            emit_weight_dmas()
            for g in range(NG_PER_B):
                for tt in range(2):
                    tn = 2 * (g + 1) + tt
                    if tn < 16:
                        xpre[tn] = emit_x_tile(tn)
                emit_A_group(0, g)

            # attention(b0) interleaved with QKV(b1)
            for qt in range(QT_PER_B):
                if qt % 2 == 0:
                    for tt in range(2):
                        xpre[16 + qt + tt] = emit_x_tile(16 + qt + tt)
                else:
                    emit_A_group(1, qt // 2)
                exs = [emit_B_S(0, qt, lh) for lh in range(HL)]
                for lh in range(HL):
                    emit_B_AV(0, qt, lh, exs[lh])
                if qt == 1:
                    nc.scalar.dma_start(out=wo_sb[:], in_=wo_d[:, :, :])
                if qt == 3:
                    for j in range(NCHUNK):
                        emit_xres(j)
            emit_collective(0)

            # attention(b1) interleaved with chunk 0/1 post-processing
            for qt in range(QT_PER_B - 1, -1, -1):
                exs = [emit_B_S(1, qt, lh) for lh in range(HL)]
                for lh in range(HL):
                    emit_B_AV(1, qt, lh, exs[lh])
                if qt == 7:
                    emit_C1(0)
                if qt == 6:
                    emit_C2(0)
                if qt == 5:
                    emit_C1(1)
                if qt == 4:
                    emit_C2(1)
            emit_collective(1)

            # early MLP1: fp8 tiles, first token half (chunks 0,1) -- fills the
            # AllToAll latency window; their W1 tiles are streamed again for
            # the second half (cheap: 5MB fp8)
            for mm in range(N1F):
                emit_D_tile(mm, 0, 256)
            emit_C1(2)
            emit_C2(2)
            emit_C1(3)
            emit_C2(3)
            # now the kq region is dead (attention complete) -> bf16 u tiles
            ubf = p_kvu.tile([P, MMT - N2F, 512], BF16, tag="kvu", name="ubf")
            for mm in range(N1F):
                emit_D_tile(mm, 256, 256)
            for mm in range(N1F, MMT):
                emit_D_tile(mm, 0, 512)
            for p in range(NPASS):
                emit_E_pass(p)
    nc.compile()
    return nc


def _bf16(a):
    return np.asarray(a, dtype=np.float32).astype(ml_dtypes.bfloat16)


def _fp8(a):
    return np.clip(np.asarray(a, np.float32), -240, 240).astype(mybir.dt.np(FP8))


def make_in_maps(x, Wq, Wk, Wv, Wo, g1, bn1, g2, bn2, W1, b1, W2, b2):
    x = np.asarray(x, np.float32)
    x_flat = np.ascontiguousarray(x.reshape(NTOK, H))

    wq_eff = (g1[:, None] * np.asarray(Wq, np.float32)) * WSCALE
    wk_eff = (g1[:, None] * np.asarray(Wk, np.float32)) * WSCALE
    wv_eff = (g1[:, None] * np.asarray(Wv, np.float32)) * WSCALE
    bq = (bn1 @ np.asarray(Wq, np.float32)) * WSCALE
    bk = (bn1 @ np.asarray(Wk, np.float32)) * WSCALE
    bv = (bn1 @ np.asarray(Wv, np.float32)) * WSCALE
    w1_eff = g2[:, None] * np.asarray(W1, np.float32)
    b1_eff = np.asarray(b1, np.float32) + bn2 @ np.asarray(W1, np.float32)

    xbf = np.ascontiguousarray(_fp8(x_flat))
    # W1: [mm, p, ks, mw]; tiles 0..N1F-1 fp8 (x16), rest bf16
    w1_t = _bf16(w1_eff).astype(np.float32).reshape(KS, P, MMT, P).transpose(2, 1, 0, 3)
    w1f8 = np.ascontiguousarray(_fp8(16.0 * w1_eff.reshape(KS, P, MMT, P)
                                     .transpose(2, 1, 0, 3)[:N1F]))
    w1bf = np.ascontiguousarray(_bf16(w1_eff.reshape(KS, P, MMT, P)
                                      .transpose(2, 1, 0, 3)[N1F:]))
    # W2 scaled by 32 on both dtypes; [pass][unit][...]
    W2f = np.asarray(W2, np.float32) * 16.0
    w2f8 = np.empty((NPASS, N2P, P, 2, 512), mybir.dt.np(FP8))
    w2bf = np.empty((NPASS, NBF2, P, 512), ml_dtypes.bfloat16)
    for p in range(NPASS):
        cols = slice(512 * p, 512 * (p + 1))
        for q in range(N2P):
            w2f8[p, q, :, 0, :] = _fp8(W2f[P * 2 * q:P * (2 * q + 1), cols])
            w2f8[p, q, :, 1, :] = _fp8(W2f[P * (2 * q + 1):P * (2 * q + 2), cols])
        for i in range(NBF2):
            mm = N2F + i
            w2bf[p, i] = _bf16(W2f[P * mm:P * (mm + 1), cols])
    b1m = np.ascontiguousarray(b1_eff.reshape(MMT, P).T.astype(np.float32))
    wo8 = np.ascontiguousarray(
        _fp8(16.0 * np.asarray(Wo, np.float32)).reshape(KS, P, H).transpose(1, 0, 2))
    ii, jj_ = np.meshgrid(np.arange(P), np.arange(P), indexing="ij")
    cmaskT = np.where(ii <= jj_, 0.0, NEG).astype(np.float32)
    b2f = np.asarray(b2, np.float32)

    in_maps = []
    for c in range(NCORES):
        cs = slice(DV * c, DV * (c + 1))
        wqk = np.concatenate([wq_eff[:, cs], wk_eff[:, cs]], axis=1)  # [H, 512]
        wqk_t = np.ascontiguousarray(
            _fp8(wqk).reshape(KS, P, DQK).transpose(1, 0, 2))
        bqk = np.concatenate([bq[cs], bk[cs]]).astype(np.float32)
        bqk_m = np.ascontiguousarray(bqk.reshape(DQK // P, P).T)
        wv_t = np.ascontiguousarray(
            _fp8(wv_eff[:, cs]).reshape(KS, P, DV).transpose(1, 0, 2))
        bvbc = np.ascontiguousarray(np.broadcast_to(
            bv[cs].astype(np.float32).reshape(1, HL, P), (P, HL, P)))
        xres = np.concatenate(
            [x_flat[1024 * j + P * c:1024 * j + P * (c + 1)] for j in range(NCHUNK)],
            axis=0) + b2f
        xres16 = np.ascontiguousarray((16.0 * xres).astype(np.float32))
        in_maps.append({
            "xbf": xbf, "xres": xres16,
            "wqk": wqk_t, "bqk": bqk_m, "wv": wv_t, "bvbc": bvbc, "wo": wo8,
            "w1f8": w1f8, "w1bf": w1bf, "b1": b1m, "w2f8": w2f8, "w2bf": w2bf,
            "cmaskT": cmaskT,
        })
    return in_maps


_NC_CACHE = {}


def kernel(**inputs):
    if "nc" not in _NC_CACHE:
        _NC_CACHE["nc"] = build()
    nc = _NC_CACHE["nc"]
    in_maps = make_in_maps(
        inputs["x"], inputs["Wq"], inputs["Wk"], inputs["Wv"], inputs["Wo"],
        np.asarray(inputs["g1"], np.float32), np.asarray(inputs["bn1"], np.float32),
        np.asarray(inputs["g2"], np.float32), np.asarray(inputs["bn2"], np.float32),
        inputs["W1"], inputs["b1"], inputs["W2"], inputs["b2"])
    res = run_bass_kernel_spmd(nc, in_maps, list(range(NCORES)))
    out = np.empty((NTOK, H), np.float32)
    for c in range(NCORES):
        oc = np.asarray(res.results[c]["out"], dtype=np.float32)
        for j in range(NCHUNK):
            out[1024 * j + P * c:1024 * j + P * (c + 1)] = oc[P * j:P * (j + 1)]
    return out.reshape(B, T, H)


# revision 21
# speedup vs baseline: 1.3551x; 1.0734x over previous
"""Fused transformer block (LN -> causal MHA -> residual -> LN -> SiLU MLP -> residual)
on 8 Trainium2 NeuronCores.

v3 design (on top of the v2 baseline):
- Tensor-parallel over heads (2 heads/core) for QKV + attention; S computed
  transposed (S^T) with the softmax denominator as a ones-column of V.
- Attention outputs are TRANSPOSED and quantized to fp8 BEFORE the AllToAll
  (payload halves to 512KB/group; after the exchange the received buffer is
  directly the lhsT of the O-projection -> no post-collective PE work beyond
  the matmuls themselves).
- x2 (attention residual) is kept in SBUF in f32, scaled by 32 so that the
  fp8(32*W2) and bf16(32*W2) MLP2 products accumulate uniformly; the final
  drain rescales by 1/32. LN2 is scale-invariant (eps scaled to match).
- MLP is FUSED: u = silu(mlp1) stays in SBUF (aliased onto the dead K/Q
  SBUF region); no DRAM round trip. MLP2 runs in 4 feature passes of 4 PSUM
  banks each.
- Mixed-precision MLP: mid-tiles 0..19 of MLP1 and mid-tile pairs 0..10 of
  MLP2 run fp8+DoubleRow (1.8x); the rest bf16. Chosen so the predicted
  rel-err (numpy-emulated, matches HW to 3 digits) is ~1.8e-2 < 2e-2.
- The first 20 MLP1 tiles run as two 256-token halves so their first halves
  (token chunks 0,1, available right after C2(1)) fill the ~50us AllToAll #2
  latency window; their fp8 W1 tiles are streamed twice (5MB extra).
- PSUM ring decoupling: attention (ring qs/sm), C (own ring c), so the
  collective-dependent O-projection never blocks attention PSUM reuse.
- Output written bf16 (host upcasts), xres folded with b2 and pre-scaled on
  the host.
"""

import sys
import os

for _p in ("/opt/trn_rl_repo", "/root/.axon_site/_ro/trn_rl_repo"):
    if os.path.isdir(_p) and _p not in sys.path:
        sys.path.insert(0, _p)
        break

import numpy as np
import ml_dtypes

import concourse.bass as bass
from concourse import bacc
import concourse.mybir as mybir
import concourse.tile as tile
from concourse.masks import make_identity
from concourse.bass_utils import run_bass_kernel_spmd

F32 = mybir.dt.float32
BF16 = mybir.dt.bfloat16
FP8 = mybir.dt.float8e4


def _install_act_table_hint():
    """Steer the act-table-set chooser so Exp and Ln resolve to the one set
    that contains BOTH (natural_log_exp_and_others)."""
    import concourse.bacc as _bacc
    if getattr(_bacc, "_act_hint_installed", False):
        return
    _orig = _bacc.get_activation_tables

    def _patched(arch):
        tabs = _orig(arch)
        exp = mybir.ActivationFunctionType.Exp
        ln = mybir.ActivationFunctionType.Ln
        for name, fns in tabs.items():
            if name != "natural_log_exp_and_others":
                fns.discard(exp)
                fns.discard(ln)
        return tabs

    _bacc.get_activation_tables = _patched
    _bacc._act_hint_installed = True


_install_act_table_hint()

P = 128          # partitions / head_dim / token tile
H = 2048         # hidden
KS = H // P      # 16 k-subtiles over hidden
HEADS = 16
HL = 2           # heads per core
NCORES = 8
B = 2
T = 2048
NTOK = B * T     # 4096
TPB = T          # tokens per batch
MID = 4 * H      # 8192
MMT = MID // P   # 64 m-tiles over mid dim
DQK = 2 * HL * P   # 512 rows of fused QK projection per core
DV = HL * P        # 256 V/attention-out features per core
EPS = 1e-5
NEG = -1.0e30

QT_PER_B = TPB // P   # 16 q tiles per batch
MT = NTOK // P        # 32 token m-tiles
NCHUNK = 4            # token chunks per core (128 each)
GT = 256              # tokens per A-group
WSCALE = 16.0         # fp8 weight rescale (avoids e4m3 subnormals)
SCORE_SCALE = (1.0 / np.sqrt(P)) / (WSCALE * WSCALE)
NG_PER_B = TPB // GT  # 8 A-groups per batch

# ---- mixed-precision MLP config ----
N1F = 20              # MLP1 mid-tiles 0..N1F-1 in fp8 DoubleRow (also the
                      # "early" tiles run as two 256-token halves)
N2P = 11              # MLP2 mid-tile PAIRS 0..N2P-1 (tiles 0..21) in fp8 DR
N2F = 2 * N2P         # fp8 MLP2 tiles
NBF2 = MMT - N2F      # 42 bf16 MLP2 tiles
NPASS = 4             # MLP2 feature passes (512 cols each)
X2S = 16.0            # x2 kept as 16*x2_true in SBUF


def build(sim=False, trn_kwargs=None, trace_sim=False):
    nc = bacc.Bacc(None, num_devices=NCORES, **(trn_kwargs or {}))

    x_d = nc.declare_dram_parameter("xbf", [NTOK, H], FP8, isOutput=False)
    xres_d = nc.declare_dram_parameter("xres", [NCHUNK * P, H], F32, isOutput=False)
    wqk_d = nc.declare_dram_parameter("wqk", [P, KS, DQK], FP8, isOutput=False)
    bqk_d = nc.declare_dram_parameter("bqk", [P, DQK // P], F32, isOutput=False)
    wv_d = nc.declare_dram_parameter("wv", [P, KS, DV], FP8, isOutput=False)
    bvbc_d = nc.declare_dram_parameter("bvbc", [P, HL, P], F32, isOutput=False)
    wo_d = nc.declare_dram_parameter("wo", [P, KS, H], FP8, isOutput=False)
    w1f8_d = nc.declare_dram_parameter("w1f8", [N1F, P, KS, P], FP8, isOutput=False)
    w1bf_d = nc.declare_dram_parameter("w1bf", [MMT - N1F, P, KS, P], BF16,
                                       isOutput=False)
    b1_d = nc.declare_dram_parameter("b1", [P, MMT], F32, isOutput=False)
    w2f8_d = nc.declare_dram_parameter("w2f8", [NPASS, N2P, P, 2, 512], FP8,
                                       isOutput=False)
    w2bf_d = nc.declare_dram_parameter("w2bf", [NPASS, NBF2, P, 512], BF16,
                                       isOutput=False)
    cmaskT_d = nc.declare_dram_parameter("cmaskT", [P, P], F32, isOutput=False)
    out_d = nc.declare_dram_parameter("out", [NCHUNK * P, H], BF16, isOutput=True)

    from contextlib import ExitStack
    with tile.TileContext(nc, trace_sim=trace_sim) as tc:
        with ExitStack() as stack:
            dram = stack.enter_context(tc.tile_pool(name="dram", bufs=1, space="DRAM"))
            const = stack.enter_context(tc.tile_pool(name="const", bufs=1))
            wbig = stack.enter_context(tc.tile_pool(name="wbig", bufs=1))
            # wqk (8KB/part, dead after QKV) chained with h2T (16KB)
            p_ali = stack.enter_context(tc.tile_pool(name="ali16", bufs=1))
            # ksb+qT (32KB, dead after last AV) chained with ubf (42KB)
            p_kvu = stack.enter_context(tc.tile_pool(name="kvu", bufs=1))
            p_vsb = stack.enter_context(tc.tile_pool(name="vsb", bufs=2))
            p_u8 = stack.enter_context(tc.tile_pool(name="u8", bufs=1))
            p_x = stack.enter_context(tc.tile_pool(name="xin", bufs=2))
            p_ln = stack.enter_context(tc.tile_pool(name="lnsmall", bufs=2))
            p_h = stack.enter_context(tc.tile_pool(name="htok", bufs=2))
            p_hT = stack.enter_context(tc.tile_pool(name="hT", bufs=2))
            p_h2T8 = stack.enter_context(tc.tile_pool(name="h2T8", bufs=1))
            p_ex = stack.enter_context(tc.tile_pool(name="expT", bufs=2))
            p_ao = stack.enter_context(tc.tile_pool(name="aot", bufs=2))
            p_aoT = stack.enter_context(tc.tile_pool(name="aoT", bufs=1))
            p_afT = stack.enter_context(tc.tile_pool(name="afT", bufs=1))
            p_x2 = stack.enter_context(tc.tile_pool(name="x2", bufs=4))
            p_w1 = stack.enter_context(tc.tile_pool(name="w1pool", bufs=2))
            p_w2 = stack.enter_context(tc.tile_pool(name="w2pool", bufs=2))
            # PSUM rings:
            #  qs: 4 x 2KB  (A-QK, A-transposes, B-S, D-full psU, E-psY)
            #  sm: 4 x 1KB  (A-V, B-AV, B-aoT transposes, D-early psU halves)
            #  c : 2 x 2KB  (C O-proj, C2 h2 transposes) -- collective-coupled
            ps_qs = stack.enter_context(tc.tile_pool(name="psqs", bufs=4, space="PSUM"))
            ps_sm = stack.enter_context(tc.tile_pool(name="pssm", bufs=2, space="PSUM"))
            ps_c = stack.enter_context(tc.tile_pool(name="psc", bufs=2, space="PSUM"))

            # ---- internal DRAM ----
            # aotT laid [g][dst s][jj][fsub][f][t]; per-(g,s) shard contiguous
            aot_dram = dram.tile([2, NCORES * 2 * 2 * P, P], BF16)
            a2a_dram = dram.tile([2, NCORES * 2 * 2 * P, P], BF16)

            # ---- constants / weights in SBUF ----
            ident = const.tile([P, P], BF16)
            make_identity(nc, ident)
            epsb = const.tile([P, 1], F32)
            nc.vector.memset(epsb[:], EPS)
            epsb2 = const.tile([P, 1], F32)
            nc.vector.memset(epsb2[:], EPS * X2S * X2S)
            scrap = const.tile([P, 1], F32)
            cmaskT = const.tile([P, P], F32)
            nc.sync.dma_start(cmaskT[:], cmaskT_d[:, :])
            bqk_sb = const.tile([P, DQK // P], F32)
            nc.sync.dma_start(bqk_sb[:], bqk_d[:, :])
            bvbc_sb = const.tile([P, HL, P], F32)
            nc.sync.dma_start(bvbc_sb[:], bvbc_d[:, :, :])
            b1_sb = const.tile([P, MMT], F32)
            nc.sync.dma_start(b1_sb[:], b1_d[:, :])
            wqk_sb = p_ali.tile([P, KS, DQK], FP8, tag="ali16", name="wqk_sb")
            wv_sb = wbig.tile([P, KS, DV], FP8)
            wo_sb = wbig.tile([P, KS, H], FP8)

            def emit_weight_dmas():
                nc.gpsimd.dma_start(out=wqk_sb[:, :KS // 2, :],
                                    in_=wqk_d[:, :KS // 2, :])
                nc.scalar.dma_start(out=wqk_sb[:, KS // 2:, :],
                                    in_=wqk_d[:, KS // 2:, :])
                nc.gpsimd.dma_start(out=wv_sb[:], in_=wv_d[:, :, :])

            def layer_norm_stats(parts, name, tagp="", eps=None):
                """parts: list of (tile, ncols512). Returns (nmu, rstd) [P,1]."""
                st = p_ln.tile([P, 4, 6], F32, tag=tagp + "lnst", name=f"st_{name}")
                a = 0
                for tile_, n in parts:
                    for i in range(n):
                        nc.vector.bn_stats(st[:, a, :], tile_[:, 512 * i:512 * (i + 1)])
                        a += 1
                assert a == 4
                mv = p_ln.tile([P, 2], F32, tag=tagp + "lnmv", name=f"mv_{name}")
                nc.vector.bn_aggr(mv[:], st[:])
                lv = p_ln.tile([P, 1], F32, tag=tagp + "lnsd", name=f"lv_{name}")
                nc.scalar.activation(lv[:], mv[:, 1:2],
                                     mybir.ActivationFunctionType.Ln,
                                     bias=(eps if eps is not None else epsb)[:])
                rstd = p_ln.tile([P, 1], F32, tag=tagp + "lnrstd", name=f"rstd_{name}")
                nc.scalar.activation(rstd[:], lv[:],
                                     mybir.ActivationFunctionType.Exp, scale=-0.5)
                nmu = p_ln.tile([P, 1], F32, tag=tagp + "lnnmu", name=f"nmu_{name}")
                nc.vector.tensor_scalar_mul(nmu[:], mv[:, 0:1], -1.0)
                return nmu[:], rstd[:]

            def ln_apply(dst, src, nmu, rstd, engine):
                engine.tensor_scalar(dst, src, nmu, rstd,
                                     mybir.AluOpType.add, mybir.AluOpType.mult)

            # ================= Stage A: LN1, transpose, QKV ===================
            # ksb/qT live in one big tile so the whole region can be reused by
            # the bf16 u tiles of the fused MLP once attention is done.
            kq_all = p_kvu.tile([P, 2, 2, HL, TPB], BF16, tag="kvu", name="kq_all")

            def ksb(b):
                return kq_all[:, b, 0]

            def qT(b):
                return kq_all[:, b, 1]

            vsb = [None, None]
            xpre = {}

            def emit_x_tile(t):
                xh = []
                for hh in range(2):
                    xth = p_x.tile([P, H // 2], FP8, tag="xt",
                                   name=f"xt_{t}_{hh}", bufs=6)
                    (nc.sync if hh == 0 else nc.scalar).dma_start(
                        out=xth[:], in_=x_d[P * t:P * (t + 1),
                                           (H // 2) * hh:(H // 2) * (hh + 1)])
                    xh.append(xth)
                return xh

            def emit_A_group(b, g):
                """LN1 + transpose + QKV for GT=256 tokens (group g of batch b)."""
                if g == 0:
                    vsb[b] = p_vsb.tile([P, QT_PER_B, HL, P + 2], BF16, tag="vsb",
                                        name=f"vsb_{b}")
                    nc.vector.memset(vsb[b][:, :, :, P:P + 1], 1.0)
                hT = p_hT.tile([P, KS, GT], FP8, tag="hT", name=f"hT_{b}_{g}")
                if b == 0:
                    ev_copy = lambda out, in_: nc.scalar.copy(out=out, in_=in_)
                    ev_bias = lambda out, in_, s: nc.scalar.add(out, in_, s)
                else:
                    ev_copy = lambda out, in_: nc.vector.tensor_copy(out=out, in_=in_)
                    ev_bias = lambda out, in_, s: nc.vector.tensor_scalar_add(
                        out, in_, s)
                for tt in range(GT // P):   # 128-token LN tiles
                    t = (TPB * b + GT * g) // P + tt
                    xh = xpre.pop(t, None)
                    if xh is None:
                        xh = emit_x_tile(t)
                    nmu, rstd = layer_norm_stats([(xh[0], 2), (xh[1], 2)],
                                                 f"ln1_{t}")
                    ht = p_h.tile([P, H], BF16, tag="ht", name=f"ht_{t}", bufs=2)
                    for hh in range(2):
                        ln_apply(ht[:, (H // 2) * hh:(H // 2) * (hh + 1)],
                                 xh[hh][:], nmu, rstd, nc.gpsimd)
                    for fg in range(KS // 8):
                        ptp = ps_c.tile([P, 1024], BF16, tag="psc",
                                        name=f"trp_{t}_{fg}")
                        for f4 in range(8):
                            f = 8 * fg + f4
                            nc.tensor.transpose(ptp[:, P * f4:P * (f4 + 1)],
                                                ht[:, P * f:P * (f + 1)], ident[:])
                        ev_copy(hT[:, 8 * fg:8 * (fg + 1), P * tt:P * (tt + 1)],
                                ptp[:].rearrange("p (a b) -> p a b", b=P))

                col0 = GT * g
                # QK projection: m 0,1 -> Q head0/1 ; 2,3 -> K head0/1
                for m in range(4):
                    ps = ps_qs.tile([P, GT], F32, tag="psqs", name=f"qk_{b}_{g}_{m}")
                    for k2 in range(KS // 2):
                        nc.tensor.matmul(
                            ps[:], lhsT=wqk_sb[:, 2 * k2:2 * k2 + 2, P * m:P * (m + 1)],
                            rhs=hT[:, 2 * k2:2 * k2 + 2, :],
                            perf_mode=mybir.MatmulPerfMode.DoubleRow,
                            start=(k2 == 0), stop=(k2 == KS // 2 - 1))
                    dst = qT(b) if m < 2 else ksb(b)
                    ev_bias(dst[:, m % 2, col0:col0 + GT], ps[:],
                            bqk_sb[:, m:m + 1])
                # V projection (token-major)
                for m in range(GT // P):
                    ps = ps_sm.tile([P, DV], F32, tag="pssm", name=f"v_{b}_{g}_{m}")
                    for k2 in range(KS // 2):
                        nc.tensor.matmul(
                            ps[:], lhsT=hT[:, 2 * k2:2 * k2 + 2, P * m:P * (m + 1)],
                            rhs=wv_sb[:, 2 * k2:2 * k2 + 2, :],
                            perf_mode=mybir.MatmulPerfMode.DoubleRow,
                            start=(k2 == 0), stop=(k2 == KS // 2 - 1))
                    tm = (GT * g) // P + m
                    nc.vector.tensor_tensor(
                        vsb[b][:, tm, :, 0:P],
                        ps[:].rearrange("p (a b) -> p a b", b=P),
                        bvbc_sb[:], mybir.AluOpType.add)

            # ================= Stage B: attention (S^T form) ==================
            aosb = {}

            def emit_B_S(b, qt, lh):
                """S^T matmuls + mask + exp for (batch, query tile, local head)."""
                klen = P * (qt + 1)
                nchs = (qt + 4) // 4
                ex = p_ex.tile([P, TPB], BF16, tag="ex", name=f"ex_{b}_{qt}_{lh}")
                qcols = qT(b)[:, lh, P * qt:P * (qt + 1)]
                for j in range(nchs):
                    n0 = 512 * j
                    n1 = min(n0 + 512, klen)
                    ps = ps_qs.tile([P, 512], F32, tag="psqs",
                                    name=f"s_{b}_{qt}_{lh}_{j}")
                    for kb in range(n0 // P, n1 // P):
                        nc.tensor.matmul(ps[:, P * kb - n0:P * (kb + 1) - n0],
                                         lhsT=ksb(b)[:, lh, P * kb:P * (kb + 1)],
                                         rhs=qcols, start=True, stop=True)
                    if j == nchs - 1:
                        d0 = klen - P - n0
                        nc.vector.tensor_tensor(ps[:, d0:d0 + P], ps[:, d0:d0 + P],
                                                cmaskT[:], mybir.AluOpType.add)
                    nc.scalar.activation(ex[:, n0:n1], ps[:, :n1 - n0],
                                         mybir.ActivationFunctionType.Exp,
                                         scale=float(SCORE_SCALE))
                return ex

            def emit_B_AV(b, qt, lh, ex):
                """A@V with ones-column, normalize; transpose + fp8-stage after
                lh=1 so the a2a payload is already in O-projection lhsT form."""
                mt = QT_PER_B * b + qt
                if lh == 0:
                    aosb[mt] = p_ao.tile([P, HL, P], BF16, tag="aot", name=f"ao_{mt}")
                psO = ps_sm.tile([P, P + 2], F32, tag="pssm", name=f"o_{mt}_{lh}")
                for kb in range(qt + 1):
                    nc.tensor.matmul(psO[:, :P + 1],
                                     lhsT=ex[:, P * kb:P * (kb + 1)],
                                     rhs=vsb[b][:, kb, lh, 0:P + 1],
                                     start=(kb == 0), stop=(kb == qt))
                rinv = p_ln.tile([P, 1], F32, tag="rinv", name=f"ri_{mt}_{lh}")
                nc.vector.reciprocal(rinv[:], psO[:, P:P + 1])
                # aosb = attn_true (v carries the 16x weight scale; /16 here)
                nc.vector.tensor_scalar(aosb[mt][:, lh, :], psO[:, 0:P],
                                        rinv[:], 1.0 / 16.0,
                                        mybir.AluOpType.mult, mybir.AluOpType.mult)
                if lh == HL - 1:
                    # transpose [tok, 2*128f] -> [2, 128f, tok], cast fp8, stage
                    ptp = ps_sm.tile([P, HL * P], BF16, tag="pssm",
                                     name=f"aop_{mt}")
                    for hh in range(HL):
                        nc.tensor.transpose(ptp[:, P * hh:P * (hh + 1)],
                                            aosb[mt][:, hh, :], ident[:])
                    aoT = p_aoT.tile([P, HL, P], BF16, tag="aoT", name=f"aoT_{mt}")
                    nc.scalar.copy(
                        out=aoT[:],
                        in_=ptp[:].rearrange("p (a b) -> p a b", b=P))
                    g, s, jj = mt // 16, mt % 8, (mt // 8) % 2
                    r0 = 512 * s + 256 * jj
                    nc.sync.dma_start(
                        aot_dram[g, r0:r0 + 256, :].rearrange(
                            "(a p) t -> p a t", a=HL),
                        aoT[:])
                    del aosb[mt]

            rg = [list(range(NCORES))]

            def emit_collective(g):
                nc.gpsimd.collective_compute(
                    "AllToAll", mybir.AluOpType.bypass, replica_groups=rg,
                    ins=[aot_dram[g, :, :]], outs=[a2a_dram[g, :, :]])

            # ================= Stage C: O-proj + LN2 per chunk ================
            h2T = p_ali.tile([P, KS, NCHUNK * P], BF16, tag="ali16", name="h2T")
            h2T8 = p_h2T8.tile([P, KS, NCHUNK * P], FP8, tag="h2T8", name="h2T8")
            x2t = [None] * NCHUNK
            c_state = {}

            def emit_xres(j):
                """xres (host: 16*(x+b2), f32) lands directly in the x2 tile."""
                x2t[j] = p_x2.tile([P, H], F32, tag="x2keep", name=f"x2_{j}")
                nc.scalar.dma_start(x2t[j][:], xres_d[P * j:P * (j + 1), :])

            def emit_C1(j):
                """a2a readback (already fp8 lhsT) + O-proj + scaled residual."""
                g, jj = j // 2, j % 2
                afT = p_afT.tile([P, KS, P], FP8, tag="afT", name=f"afT_{j}")
                a2av = a2a_dram[g].rearrange("(s j f p) t -> p s j f t",
                                             s=NCORES, j=2, p=P)
                for fs in range(2):
                    nc.gpsimd.dma_start(
                        out=afT[:].rearrange("p (s f) t -> p s f t",
                                             s=NCORES)[:, :, fs],
                        in_=a2av[:, :, jj, fs])
                x2 = x2t[j]
                for nn in range(4):
                    psn = ps_c.tile([P, 512], F32, tag="psc", name=f"op_{j}_{nn}")
                    for k2 in range(KS // 2):
                        nc.tensor.matmul(
                            psn[:], lhsT=afT[:, 2 * k2:2 * k2 + 2, :],
                            rhs=wo_sb[:, 2 * k2:2 * k2 + 2, 512 * nn:512 * (nn + 1)],
                            perf_mode=mybir.MatmulPerfMode.DoubleRow,
                            start=(k2 == 0), stop=(k2 == KS // 2 - 1))
                    c0 = 512 * nn
                    # x2 = psO2 + 16*(x+b2)  -> 16 * x2_true   (in place)
                    nc.vector.tensor_tensor(
                        x2[:, c0:c0 + 512], psn[:], x2[:, c0:c0 + 512],
                        mybir.AluOpType.add)
                nmu, rstd = layer_norm_stats([(x2, 4)], f"ln2_{j}", tagp="c2",
                                             eps=epsb2)
                c_state[j] = (x2, nmu, rstd)

            def emit_C2(j):
                """LN2 apply + h2 transpose into h2T (bf16) and h2T8 (fp8)."""
                x2, nmu, rstd = c_state.pop(j)
                h2 = p_h.tile([P, H], BF16, tag="ht", name=f"h2_{j}", bufs=2)
                for hh in range(2):
                    ln_apply(h2[:, (H // 2) * hh:(H // 2) * (hh + 1)],
                             x2[:, (H // 2) * hh:(H // 2) * (hh + 1)],
                             nmu, rstd, nc.gpsimd)
                for fg in range(KS // 8):
                    ptp = ps_c.tile([P, 1024], BF16, tag="psc", name=f"h2t_{j}_{fg}")
                    for f4 in range(8):
                        f = 8 * fg + f4
                        nc.tensor.transpose(ptp[:, P * f4:P * (f4 + 1)],
                                            h2[:, P * f:P * (f + 1)], ident[:])
                    nc.vector.tensor_copy(
                        out=h2T[:, 8 * fg:8 * (fg + 1), P * j:P * (j + 1)],
                        in_=ptp[:].rearrange("p (a b) -> p a b", b=P))
                    nc.scalar.copy(
                        out=h2T8[:, 8 * fg:8 * (fg + 1), P * j:P * (j + 1)],
                        in_=ptp[:].rearrange("p (a b) -> p a b", b=P))

            # ================= Stage D: MLP1 (fused, u stays in SBUF) =========
            silu_fn = (mybir.ActivationFunctionType.Sigmoid if sim
                       else mybir.ActivationFunctionType.Silu)
            # u8p: fp8 mid-tiles 0..N2F-1 as DoubleRow pairs; ubf: tiles N2F..63
            u8p = p_u8.tile([P, N2P, 2, 512], FP8, tag="u8p", name="u8p")
            ubf = None   # allocated after attention frees kq_all

            def u_dst(mm, c0, cn):
                if mm < N2F:
                    return u8p[:, mm // 2, mm % 2, c0:c0 + cn]
                return ubf[:, mm - N2F, c0:c0 + cn]

            def emit_D_tile(mm, c0, cn, w1t=None):
                """MLP1 mid-tile mm over token cols [c0, c0+cn)."""
                fp8 = mm < N1F
                if w1t is None:
                    pool, tg = ((p_w1, "w1t") if mm % 2 == 0 else (p_hT, "hT"))
                    q = nc.gpsimd if mm % 2 == 0 else nc.sync
                    if fp8:
                        w1t = pool.tile([P, KS, P], FP8, tag=tg,
                                        name=f"w1t_{mm}_{c0}")
                        q.dma_start(out=w1t[:], in_=w1f8_d[mm, :, :, :])
                    else:
                        w1t = pool.tile([P, KS, P], BF16, tag=tg,
                                        name=f"w1t_{mm}_{c0}")
                        q.dma_start(out=w1t[:], in_=w1bf_d[mm - N1F, :, :, :])
                if cn == 512:
                    ps = ps_qs.tile([P, 512], F32, tag="psqs", name=f"u_{mm}")
                else:
                    ps = ps_qs.tile([P, cn], F32, tag="psqs", name=f"u_{mm}_{c0}")
                if fp8:
                    for k2 in range(KS // 2):
                        nc.tensor.matmul(
                            ps[:], lhsT=w1t[:, 2 * k2:2 * k2 + 2, :],
                            rhs=h2T8[:, 2 * k2:2 * k2 + 2, c0:c0 + cn],
                            perf_mode=mybir.MatmulPerfMode.DoubleRow,
                            start=(k2 == 0), stop=(k2 == KS // 2 - 1))
                    sc = 1.0 / 16.0
                else:
                    for ks in range(KS):
                        nc.tensor.matmul(ps[:], lhsT=w1t[:, ks, :],
                                         rhs=h2T[:, ks, c0:c0 + cn],
                                         start=(ks == 0), stop=(ks == KS - 1))
                    sc = 1.0
                nc.scalar.activation(u_dst(mm, c0, cn), ps[:], silu_fn,
                                     bias=b1_sb[:, mm:mm + 1], scale=sc)
                return w1t

            # ================= Stage E: MLP2 (4 feature passes) ===============
            def emit_E_pass(p):
                psY = [ps_qs.tile([P, 512], F32, tag="psqs", name=f"y_{p}_{jj}")
                       for jj in range(4)]
                nunit = N2P + NBF2
                for un in range(nunit):
                    pool, tg = ((p_w2, "w2t") if un % 2 == 0 else (p_vsb, "vsb"))
                    q = nc.sync if un % 2 == 0 else nc.scalar
                    if un < N2P:
                        w2t = pool.tile([P, 2, 512], FP8, tag=tg,
                                        name=f"w2t_{p}_{un}")
                        q.dma_start(out=w2t[:], in_=w2f8_d[p, un, :, :, :])
                        for jj in range(4):
                            nc.tensor.matmul(
                                psY[jj][:],
                                lhsT=u8p[:, un, :, P * jj:P * (jj + 1)],
                                rhs=w2t[:],
                                perf_mode=mybir.MatmulPerfMode.DoubleRow,
                                start=(un == 0), stop=(un == nunit - 1))
                    else:
                        w2t = pool.tile([P, 512], BF16, tag=tg,
                                        name=f"w2t_{p}_{un}")
                        q.dma_start(out=w2t[:],
                                    in_=w2bf_d[p, un - N2P, :, :])
                        for jj in range(4):
                            nc.tensor.matmul(
                                psY[jj][:],
                                lhsT=ubf[:, un - N2P, P * jj:P * (jj + 1)],
                                rhs=w2t[:],
                                start=(un == 0), stop=(un == nunit - 1))
                for jj in range(4):
                    # out = (psY + 16*x2_true) / 16: add in psum, scaled copy
                    nc.vector.tensor_tensor(
                        psY[jj][:], psY[jj][:], x2t[jj][:, 512 * p:512 * (p + 1)],
                        mybir.AluOpType.add)
                    ot = p_x.tile([P, 512], BF16, tag="xt", name=f"ot_{p}_{jj}", bufs=6)
                    nc.scalar.activation(ot[:], psY[jj][:],
                                         mybir.ActivationFunctionType.Copy,
                                         scale=1.0 / X2S)
                    nc.scalar.dma_start(
                        out=out_d[P * jj:P * (jj + 1), 512 * p:512 * (p + 1)],
                        in_=ot[:])

            # ================= emission schedule ==============================
            for t in range(2):
                xpre[t] = emit_x_tile(t)
            emit_weight_dmas()
            for g in range(NG_PER_B):
                for tt in range(2):
                    tn = 2 * (g + 1) + tt
                    if tn < 16:
                        xpre[tn] = emit_x_tile(tn)
                emit_A_group(0, g)

            # attention(b0) interleaved with QKV(b1)
            for qt in range(QT_PER_B):
                if qt % 2 == 0:
                    for tt in range(2):
                        xpre[16 + qt + tt] = emit_x_tile(16 + qt + tt)
                else:
                    emit_A_group(1, qt // 2)
                exs = [emit_B_S(0, qt, lh) for lh in range(HL)]
                for lh in range(HL):
                    emit_B_AV(0, qt, lh, exs[lh])
                if qt == 1:
                    nc.scalar.dma_start(out=wo_sb[:], in_=wo_d[:, :, :])
                if qt == 3:
                    for j in range(NCHUNK):
                        emit_xres(j)
            emit_collective(0)

            # attention(b1) interleaved with chunk 0/1 post-processing
            w1pre = {}

            def prefetch_w1(mm):
                pool, tg = ((p_w1, "w1t") if mm % 2 == 0 else (p_hT, "hT"))
                q = nc.gpsimd if mm % 2 == 0 else nc.sync
                w1t = pool.tile([P, KS, P], FP8, tag=tg, name=f"w1p_{mm}")
                q.dma_start(out=w1t[:], in_=w1f8_d[mm, :, :, :])
                w1pre[mm] = w1t

            for qt in range(QT_PER_B):
                exs = [emit_B_S(1, qt, lh) for lh in range(HL)]
                for lh in range(HL):
                    emit_B_AV(1, qt, lh, exs[lh])
                if qt == 9:
                    emit_C1(0)
                if qt == 10:
                    emit_C2(0)
                if qt == 11:
                    emit_C1(1)
                if qt == 12:
                    emit_C2(1)
                if qt == 13:
                    for mm in range(4):
                        prefetch_w1(mm)
                if qt == 14:
                    for mm in range(4, 8):
                        prefetch_w1(mm)
            emit_collective(1)

            # early MLP1: fp8 tiles, first token half (chunks 0,1) -- fills the
            # AllToAll latency window; their W1 tiles are streamed again for
            # the second half (cheap: 5MB fp8)
            for mm in range(N1F):
                emit_D_tile(mm, 0, 256, w1t=w1pre.pop(mm, None))
            emit_C1(2)
            emit_C2(2)
            emit_C1(3)
            emit_C2(3)
            # now the kq region is dead (attention complete) -> bf16 u tiles
            ubf = p_kvu.tile([P, MMT - N2F, 512], BF16, tag="kvu", name="ubf")
            for mm in range(N1F):
                emit_D_tile(mm, 256, 256)
            for mm in range(N1F, MMT):
                emit_D_tile(mm, 0, 512)
            for p in range(NPASS):
                emit_E_pass(p)
    nc.compile()
    return nc


def _bf16(a):
    return np.asarray(a, dtype=np.float32).astype(ml_dtypes.bfloat16)


def _fp8(a):
    return np.clip(np.asarray(a, np.float32), -240, 240).astype(mybir.dt.np(FP8))


def make_in_maps(x, Wq, Wk, Wv, Wo, g1, bn1, g2, bn2, W1, b1, W2, b2):
    x = np.asarray(x, np.float32)
    x_flat = np.ascontiguousarray(x.reshape(NTOK, H))

    wq_eff = (g1[:, None] * np.asarray(Wq, np.float32)) * WSCALE
    wk_eff = (g1[:, None] * np.asarray(Wk, np.float32)) * WSCALE
    wv_eff = (g1[:, None] * np.asarray(Wv, np.float32)) * WSCALE
    bq = (bn1 @ np.asarray(Wq, np.float32)) * WSCALE
    bk = (bn1 @ np.asarray(Wk, np.float32)) * WSCALE
    bv = (bn1 @ np.asarray(Wv, np.float32)) * WSCALE
    w1_eff = g2[:, None] * np.asarray(W1, np.float32)
    b1_eff = np.asarray(b1, np.float32) + bn2 @ np.asarray(W1, np.float32)

    xbf = np.ascontiguousarray(_fp8(x_flat))
    # W1: [mm, p, ks, mw]; tiles 0..N1F-1 fp8 (x16), rest bf16
    w1_t = _bf16(w1_eff).astype(np.float32).reshape(KS, P, MMT, P).transpose(2, 1, 0, 3)
    w1f8 = np.ascontiguousarray(_fp8(16.0 * w1_eff.reshape(KS, P, MMT, P)
                                     .transpose(2, 1, 0, 3)[:N1F]))
    w1bf = np.ascontiguousarray(_bf16(w1_eff.reshape(KS, P, MMT, P)
                                      .transpose(2, 1, 0, 3)[N1F:]))
    # W2 scaled by 32 on both dtypes; [pass][unit][...]
    W2f = np.asarray(W2, np.float32) * 16.0
    w2f8 = np.empty((NPASS, N2P, P, 2, 512), mybir.dt.np(FP8))
    w2bf = np.empty((NPASS, NBF2, P, 512), ml_dtypes.bfloat16)
    for p in range(NPASS):
        cols = slice(512 * p, 512 * (p + 1))
        for q in range(N2P):
            w2f8[p, q, :, 0, :] = _fp8(W2f[P * 2 * q:P * (2 * q + 1), cols])
            w2f8[p, q, :, 1, :] = _fp8(W2f[P * (2 * q + 1):P * (2 * q + 2), cols])
        for i in range(NBF2):
            mm = N2F + i
            w2bf[p, i] = _bf16(W2f[P * mm:P * (mm + 1), cols])
    b1m = np.ascontiguousarray(b1_eff.reshape(MMT, P).T.astype(np.float32))
    wo8 = np.ascontiguousarray(
        _fp8(16.0 * np.asarray(Wo, np.float32)).reshape(KS, P, H).transpose(1, 0, 2))
    ii, jj_ = np.meshgrid(np.arange(P), np.arange(P), indexing="ij")
    cmaskT = np.where(ii <= jj_, 0.0, NEG).astype(np.float32)
    b2f = np.asarray(b2, np.float32)

    in_maps = []
    for c in range(NCORES):
        cs = slice(DV * c, DV * (c + 1))
        wqk = np.concatenate([wq_eff[:, cs], wk_eff[:, cs]], axis=1)  # [H, 512]
        wqk_t = np.ascontiguousarray(
            _fp8(wqk).reshape(KS, P, DQK).transpose(1, 0, 2))
        bqk = np.concatenate([bq[cs], bk[cs]]).astype(np.float32)
        bqk_m = np.ascontiguousarray(bqk.reshape(DQK // P, P).T)
        wv_t = np.ascontiguousarray(
            _fp8(wv_eff[:, cs]).reshape(KS, P, DV).transpose(1, 0, 2))
        bvbc = np.ascontiguousarray(np.broadcast_to(
            bv[cs].astype(np.float32).reshape(1, HL, P), (P, HL, P)))
        xres = np.concatenate(
            [x_flat[1024 * j + P * c:1024 * j + P * (c + 1)] for j in range(NCHUNK)],
            axis=0) + b2f
        xres16 = np.ascontiguousarray((16.0 * xres).astype(np.float32))
        in_maps.append({
            "xbf": xbf, "xres": xres16,
            "wqk": wqk_t, "bqk": bqk_m, "wv": wv_t, "bvbc": bvbc, "wo": wo8,
            "w1f8": w1f8, "w1bf": w1bf, "b1": b1m, "w2f8": w2f8, "w2bf": w2bf,
            "cmaskT": cmaskT,
        })
    return in_maps


_NC_CACHE = {}


def kernel(**inputs):
    if "nc" not in _NC_CACHE:
        _NC_CACHE["nc"] = build()
    nc = _NC_CACHE["nc"]
    in_maps = make_in_maps(
        inputs["x"], inputs["Wq"], inputs["Wk"], inputs["Wv"], inputs["Wo"],
        np.asarray(inputs["g1"], np.float32), np.asarray(inputs["bn1"], np.float32),
        np.asarray(inputs["g2"], np.float32), np.asarray(inputs["bn2"], np.float32),
        inputs["W1"], inputs["b1"], inputs["W2"], inputs["b2"])
    res = run_bass_kernel_spmd(nc, in_maps, list(range(NCORES)))
    out = np.empty((NTOK, H), np.float32)
    for c in range(NCORES):
        oc = np.asarray(res.results[c]["out"], dtype=np.float32)
        for j in range(NCHUNK):
            out[1024 * j + P * c:1024 * j + P * (c + 1)] = oc[P * j:P * (j + 1)]
    return out.reshape(B, T, H)


# revision 22
# speedup vs baseline: 1.3583x; 1.0023x over previous
"""Fused transformer block (LN -> causal MHA -> residual -> LN -> SiLU MLP -> residual)
on 8 Trainium2 NeuronCores.

v3 design (on top of the v2 baseline):
- Tensor-parallel over heads (2 heads/core) for QKV + attention; S computed
  transposed (S^T) with the softmax denominator as a ones-column of V.
- Attention outputs are TRANSPOSED and quantized to fp8 BEFORE the AllToAll
  (payload halves to 512KB/group; after the exchange the received buffer is
  directly the lhsT of the O-projection -> no post-collective PE work beyond
  the matmuls themselves).
- x2 (attention residual) is kept in SBUF in f32, scaled by 32 so that the
  fp8(32*W2) and bf16(32*W2) MLP2 products accumulate uniformly; the final
  drain rescales by 1/32. LN2 is scale-invariant (eps scaled to match).
- MLP is FUSED: u = silu(mlp1) stays in SBUF (aliased onto the dead K/Q
  SBUF region); no DRAM round trip. MLP2 runs in 4 feature passes of 4 PSUM
  banks each.
- Mixed-precision MLP: mid-tiles 0..19 of MLP1 and mid-tile pairs 0..10 of
  MLP2 run fp8+DoubleRow (1.8x); the rest bf16. Chosen so the predicted
  rel-err (numpy-emulated, matches HW to 3 digits) is ~1.8e-2 < 2e-2.
- The first 20 MLP1 tiles run as two 256-token halves so their first halves
  (token chunks 0,1, available right after C2(1)) fill the ~50us AllToAll #2
  latency window; their fp8 W1 tiles are streamed twice (5MB extra).
- PSUM ring decoupling: attention (ring qs/sm), C (own ring c), so the
  collective-dependent O-projection never blocks attention PSUM reuse.
- Output written bf16 (host upcasts), xres folded with b2 and pre-scaled on
  the host.
"""

import sys
import os

for _p in ("/opt/trn_rl_repo", "/root/.axon_site/_ro/trn_rl_repo"):
    if os.path.isdir(_p) and _p not in sys.path:
        sys.path.insert(0, _p)
        break

import numpy as np
import ml_dtypes

import concourse.bass as bass
from concourse import bacc
import concourse.mybir as mybir
import concourse.tile as tile
from concourse.masks import make_identity
from concourse.bass_utils import run_bass_kernel_spmd

F32 = mybir.dt.float32
BF16 = mybir.dt.bfloat16
FP8 = mybir.dt.float8e4


def _install_act_table_hint():
    """Steer the act-table-set chooser so Exp and Ln resolve to the one set
    that contains BOTH (natural_log_exp_and_others)."""
    import concourse.bacc as _bacc
    if getattr(_bacc, "_act_hint_installed", False):
        return
    _orig = _bacc.get_activation_tables

    def _patched(arch):
        tabs = _orig(arch)
        exp = mybir.ActivationFunctionType.Exp
        ln = mybir.ActivationFunctionType.Ln
        for name, fns in tabs.items():
            if name != "natural_log_exp_and_others":
                fns.discard(exp)
                fns.discard(ln)
        return tabs

    _bacc.get_activation_tables = _patched
    _bacc._act_hint_installed = True


_install_act_table_hint()

P = 128          # partitions / head_dim / token tile
H = 2048         # hidden
KS = H // P      # 16 k-subtiles over hidden
HEADS = 16
HL = 2           # heads per core
NCORES = 8
B = 2
T = 2048
NTOK = B * T     # 4096
TPB = T          # tokens per batch
MID = 4 * H      # 8192
MMT = MID // P   # 64 m-tiles over mid dim
DQK = 2 * HL * P   # 512 rows of fused QK projection per core
DV = HL * P        # 256 V/attention-out features per core
EPS = 1e-5
NEG = -1.0e30

QT_PER_B = TPB // P   # 16 q tiles per batch
MT = NTOK // P        # 32 token m-tiles
NCHUNK = 4            # token chunks per core (128 each)
GT = 256              # tokens per A-group
WSCALE = 16.0         # fp8 weight rescale (avoids e4m3 subnormals)
SCORE_SCALE = (1.0 / np.sqrt(P)) / (WSCALE * WSCALE)
NG_PER_B = TPB // GT  # 8 A-groups per batch

# ---- mixed-precision MLP config ----
N1F = 20              # MLP1 mid-tiles 0..N1F-1 in fp8 DoubleRow (also the
                      # "early" tiles run as two 256-token halves)
N2P = 11              # MLP2 mid-tile PAIRS 0..N2P-1 (tiles 0..21) in fp8 DR
N2F = 2 * N2P         # fp8 MLP2 tiles
NBF2 = MMT - N2F      # 42 bf16 MLP2 tiles
NPASS = 4             # MLP2 feature passes (512 cols each)
X2S = 16.0            # x2 kept as 16*x2_true in SBUF


def build(sim=False, trn_kwargs=None, trace_sim=False):
    nc = bacc.Bacc(None, num_devices=NCORES, **(trn_kwargs or {}))

    x_d = nc.declare_dram_parameter("xbf", [NTOK, H], FP8, isOutput=False)
    xres_d = nc.declare_dram_parameter("xres", [NCHUNK * P, H], F32, isOutput=False)
    wqk_d = nc.declare_dram_parameter("wqk", [P, KS, DQK], FP8, isOutput=False)
    bqk_d = nc.declare_dram_parameter("bqk", [P, DQK // P], F32, isOutput=False)
    wv_d = nc.declare_dram_parameter("wv", [P, KS, DV], FP8, isOutput=False)
    bvbc_d = nc.declare_dram_parameter("bvbc", [P, HL, P], F32, isOutput=False)
    wo_d = nc.declare_dram_parameter("wo", [P, KS, H], FP8, isOutput=False)
    w1f8_d = nc.declare_dram_parameter("w1f8", [N1F, P, KS, P], FP8, isOutput=False)
    w1bf_d = nc.declare_dram_parameter("w1bf", [MMT - N1F, P, KS, P], BF16,
                                       isOutput=False)
    b1_d = nc.declare_dram_parameter("b1", [P, MMT], F32, isOutput=False)
    w2f8_d = nc.declare_dram_parameter("w2f8", [NPASS, N2P, P, 2, 512], FP8,
                                       isOutput=False)
    w2bf_d = nc.declare_dram_parameter("w2bf", [NPASS, NBF2, P, 512], BF16,
                                       isOutput=False)
    cmaskT_d = nc.declare_dram_parameter("cmaskT", [P, P], F32, isOutput=False)
    out_d = nc.declare_dram_parameter("out", [NCHUNK * P, H], BF16, isOutput=True)

    from contextlib import ExitStack
    with tile.TileContext(nc, trace_sim=trace_sim) as tc:
        with ExitStack() as stack:
            dram = stack.enter_context(tc.tile_pool(name="dram", bufs=1, space="DRAM"))
            const = stack.enter_context(tc.tile_pool(name="const", bufs=1))
            wbig = stack.enter_context(tc.tile_pool(name="wbig", bufs=1))
            # wqk (8KB/part, dead after QKV) chained with h2T (16KB)
            p_ali = stack.enter_context(tc.tile_pool(name="ali16", bufs=1))
            # ksb+qT (32KB, dead after last AV) chained with ubf (42KB)
            p_kvu = stack.enter_context(tc.tile_pool(name="kvu", bufs=1))
            p_vsb = stack.enter_context(tc.tile_pool(name="vsb", bufs=2))
            p_u8 = stack.enter_context(tc.tile_pool(name="u8", bufs=1))
            p_x = stack.enter_context(tc.tile_pool(name="xin", bufs=2))
            p_ln = stack.enter_context(tc.tile_pool(name="lnsmall", bufs=2))
            p_h = stack.enter_context(tc.tile_pool(name="htok", bufs=2))
            p_hT = stack.enter_context(tc.tile_pool(name="hT", bufs=2))
            p_h2T8 = stack.enter_context(tc.tile_pool(name="h2T8", bufs=1))
            p_ex = stack.enter_context(tc.tile_pool(name="expT", bufs=2))
            p_ao = stack.enter_context(tc.tile_pool(name="aot", bufs=2))
            p_aoT = stack.enter_context(tc.tile_pool(name="aoT", bufs=1))
            p_afT = stack.enter_context(tc.tile_pool(name="afT", bufs=1))
            p_x2 = stack.enter_context(tc.tile_pool(name="x2", bufs=4))
            p_w1 = stack.enter_context(tc.tile_pool(name="w1pool", bufs=2))
            p_w2 = stack.enter_context(tc.tile_pool(name="w2pool", bufs=2))
            # PSUM rings:
            #  qs: 4 x 2KB  (A-QK, A-transposes, B-S, D-full psU, E-psY)
            #  sm: 4 x 1KB  (A-V, B-AV, B-aoT transposes, D-early psU halves)
            #  c : 2 x 2KB  (C O-proj, C2 h2 transposes) -- collective-coupled
            ps_qs = stack.enter_context(tc.tile_pool(name="psqs", bufs=4, space="PSUM"))
            ps_sm = stack.enter_context(tc.tile_pool(name="pssm", bufs=2, space="PSUM"))
            ps_c = stack.enter_context(tc.tile_pool(name="psc", bufs=2, space="PSUM"))

            # ---- internal DRAM ----
            # aotT laid [g][dst s][jj][fsub][f][t]; per-(g,s) shard contiguous
            aot_dram = dram.tile([2, NCORES * 2 * 2 * P, P], BF16)
            a2a_dram = dram.tile([2, NCORES * 2 * 2 * P, P], BF16)

            # ---- constants / weights in SBUF ----
            ident = const.tile([P, P], BF16)
            make_identity(nc, ident)
            epsb = const.tile([P, 1], F32)
            nc.vector.memset(epsb[:], EPS)
            epsb2 = const.tile([P, 1], F32)
            nc.vector.memset(epsb2[:], EPS * X2S * X2S)
            scrap = const.tile([P, 1], F32)
            cmaskT = const.tile([P, P], F32)
            nc.sync.dma_start(cmaskT[:], cmaskT_d[:, :])
            bqk_sb = const.tile([P, DQK // P], F32)
            nc.sync.dma_start(bqk_sb[:], bqk_d[:, :])
            bvbc_sb = const.tile([P, HL, P], F32)
            nc.sync.dma_start(bvbc_sb[:], bvbc_d[:, :, :])
            b1_sb = const.tile([P, MMT], F32)
            nc.sync.dma_start(b1_sb[:], b1_d[:, :])
            wqk_sb = p_ali.tile([P, KS, DQK], FP8, tag="ali16", name="wqk_sb")
            wv_sb = wbig.tile([P, KS, DV], FP8)
            wo_sb = wbig.tile([P, KS, H], FP8)

            def emit_weight_dmas():
                nc.gpsimd.dma_start(out=wqk_sb[:, :KS // 2, :],
                                    in_=wqk_d[:, :KS // 2, :])
                nc.scalar.dma_start(out=wqk_sb[:, KS // 2:, :],
                                    in_=wqk_d[:, KS // 2:, :])
                nc.gpsimd.dma_start(out=wv_sb[:], in_=wv_d[:, :, :])

            def layer_norm_stats(parts, name, tagp="", eps=None):
                """parts: list of (tile, ncols512). Returns (nmu, rstd) [P,1]."""
                st = p_ln.tile([P, 4, 6], F32, tag=tagp + "lnst", name=f"st_{name}")
                a = 0
                for tile_, n in parts:
                    for i in range(n):
                        nc.vector.bn_stats(st[:, a, :], tile_[:, 512 * i:512 * (i + 1)])
                        a += 1
                assert a == 4
                mv = p_ln.tile([P, 2], F32, tag=tagp + "lnmv", name=f"mv_{name}")
                nc.vector.bn_aggr(mv[:], st[:])
                lv = p_ln.tile([P, 1], F32, tag=tagp + "lnsd", name=f"lv_{name}")
                nc.scalar.activation(lv[:], mv[:, 1:2],
                                     mybir.ActivationFunctionType.Ln,
                                     bias=(eps if eps is not None else epsb)[:])
                rstd = p_ln.tile([P, 1], F32, tag=tagp + "lnrstd", name=f"rstd_{name}")
                nc.scalar.activation(rstd[:], lv[:],
                                     mybir.ActivationFunctionType.Exp, scale=-0.5)
                nmu = p_ln.tile([P, 1], F32, tag=tagp + "lnnmu", name=f"nmu_{name}")
                nc.vector.tensor_scalar_mul(nmu[:], mv[:, 0:1], -1.0)
                return nmu[:], rstd[:]

            def ln_apply(dst, src, nmu, rstd, engine):
                engine.tensor_scalar(dst, src, nmu, rstd,
                                     mybir.AluOpType.add, mybir.AluOpType.mult)

            # ================= Stage A: LN1, transpose, QKV ===================
            # ksb/qT live in one big tile so the whole region can be reused by
            # the bf16 u tiles of the fused MLP once attention is done.
            kq_all = p_kvu.tile([P, 2, 2, HL, TPB], BF16, tag="kvu", name="kq_all")

            def ksb(b):
                return kq_all[:, b, 0]

            def qT(b):
                return kq_all[:, b, 1]

            vsb = [None, None]
            xpre = {}

            def emit_x_tile(t):
                xh = []
                for hh in range(2):
                    xth = p_x.tile([P, H // 2], FP8, tag="xt",
                                   name=f"xt_{t}_{hh}", bufs=6)
                    (nc.sync if hh == 0 else nc.scalar).dma_start(
                        out=xth[:], in_=x_d[P * t:P * (t + 1),
                                           (H // 2) * hh:(H // 2) * (hh + 1)])
                    xh.append(xth)
                return xh

            def emit_A_group(b, g):
                """LN1 + transpose + QKV for GT=256 tokens (group g of batch b)."""
                if g == 0:
                    vsb[b] = p_vsb.tile([P, QT_PER_B, HL, P + 2], BF16, tag="vsb",
                                        name=f"vsb_{b}")
                    nc.vector.memset(vsb[b][:, :, :, P:P + 1], 1.0)
                hT = p_hT.tile([P, KS, GT], FP8, tag="hT", name=f"hT_{b}_{g}")
                if b == 0:
                    ev_copy = lambda out, in_: nc.scalar.copy(out=out, in_=in_)
                    ev_bias = lambda out, in_, s: nc.scalar.add(out, in_, s)
                else:
                    ev_copy = lambda out, in_: nc.vector.tensor_copy(out=out, in_=in_)
                    ev_bias = lambda out, in_, s: nc.vector.tensor_scalar_add(
                        out, in_, s)
                for tt in range(GT // P):   # 128-token LN tiles
                    t = (TPB * b + GT * g) // P + tt
                    xh = xpre.pop(t, None)
                    if xh is None:
                        xh = emit_x_tile(t)
                    nmu, rstd = layer_norm_stats([(xh[0], 2), (xh[1], 2)],
                                                 f"ln1_{t}")
                    ht = p_h.tile([P, H], BF16, tag="ht", name=f"ht_{t}", bufs=2)
                    for hh in range(2):
                        ln_apply(ht[:, (H // 2) * hh:(H // 2) * (hh + 1)],
                                 xh[hh][:], nmu, rstd, nc.gpsimd)
                    for fg in range(KS // 8):
                        ptp = ps_c.tile([P, 1024], BF16, tag="psc",
                                        name=f"trp_{t}_{fg}")
                        for f4 in range(8):
                            f = 8 * fg + f4
                            nc.tensor.transpose(ptp[:, P * f4:P * (f4 + 1)],
                                                ht[:, P * f:P * (f + 1)], ident[:])
                        ev_copy(hT[:, 8 * fg:8 * (fg + 1), P * tt:P * (tt + 1)],
                                ptp[:].rearrange("p (a b) -> p a b", b=P))

                col0 = GT * g
                # QK projection: m 0,1 -> Q head0/1 ; 2,3 -> K head0/1
                for m in range(4):
                    ps = ps_qs.tile([P, GT], F32, tag="psqs", name=f"qk_{b}_{g}_{m}")
                    for k2 in range(KS // 2):
                        nc.tensor.matmul(
                            ps[:], lhsT=wqk_sb[:, 2 * k2:2 * k2 + 2, P * m:P * (m + 1)],
                            rhs=hT[:, 2 * k2:2 * k2 + 2, :],
                            perf_mode=mybir.MatmulPerfMode.DoubleRow,
                            start=(k2 == 0), stop=(k2 == KS // 2 - 1))
                    dst = qT(b) if m < 2 else ksb(b)
                    ev_bias(dst[:, m % 2, col0:col0 + GT], ps[:],
                            bqk_sb[:, m:m + 1])
                # V projection (token-major)
                for m in range(GT // P):
                    ps = ps_sm.tile([P, DV], F32, tag="pssm", name=f"v_{b}_{g}_{m}")
                    for k2 in range(KS // 2):
                        nc.tensor.matmul(
                            ps[:], lhsT=hT[:, 2 * k2:2 * k2 + 2, P * m:P * (m + 1)],
                            rhs=wv_sb[:, 2 * k2:2 * k2 + 2, :],
                            perf_mode=mybir.MatmulPerfMode.DoubleRow,
                            start=(k2 == 0), stop=(k2 == KS // 2 - 1))
                    tm = (GT * g) // P + m
                    nc.vector.tensor_tensor(
                        vsb[b][:, tm, :, 0:P],
                        ps[:].rearrange("p (a b) -> p a b", b=P),
                        bvbc_sb[:], mybir.AluOpType.add)

            # ================= Stage B: attention (S^T form) ==================
            aosb = {}

            def emit_B_S(b, qt, lh):
                """S^T matmuls + mask + exp for (batch, query tile, local head)."""
                klen = P * (qt + 1)
                nchs = (qt + 4) // 4
                ex = p_ex.tile([P, TPB], BF16, tag="ex", name=f"ex_{b}_{qt}_{lh}")
                qcols = qT(b)[:, lh, P * qt:P * (qt + 1)]
                for j in range(nchs):
                    n0 = 512 * j
                    n1 = min(n0 + 512, klen)
                    ps = ps_qs.tile([P, 512], F32, tag="psqs",
                                    name=f"s_{b}_{qt}_{lh}_{j}")
                    for kb in range(n0 // P, n1 // P):
                        nc.tensor.matmul(ps[:, P * kb - n0:P * (kb + 1) - n0],
                                         lhsT=ksb(b)[:, lh, P * kb:P * (kb + 1)],
                                         rhs=qcols, start=True, stop=True)
                    if j == nchs - 1:
                        d0 = klen - P - n0
                        nc.vector.tensor_tensor(ps[:, d0:d0 + P], ps[:, d0:d0 + P],
                                                cmaskT[:], mybir.AluOpType.add)
                    nc.scalar.activation(ex[:, n0:n1], ps[:, :n1 - n0],
                                         mybir.ActivationFunctionType.Exp,
                                         scale=float(SCORE_SCALE))
                return ex

            def emit_B_AV(b, qt, lh, ex):
                """A@V with ones-column, normalize; transpose + fp8-stage after
                lh=1 so the a2a payload is already in O-projection lhsT form."""
                mt = QT_PER_B * b + qt
                if lh == 0:
                    aosb[mt] = p_ao.tile([P, HL, P], BF16, tag="aot", name=f"ao_{mt}")
                psO = ps_sm.tile([P, P + 2], F32, tag="pssm", name=f"o_{mt}_{lh}")
                for kb in range(qt + 1):
                    nc.tensor.matmul(psO[:, :P + 1],
                                     lhsT=ex[:, P * kb:P * (kb + 1)],
                                     rhs=vsb[b][:, kb, lh, 0:P + 1],
                                     start=(kb == 0), stop=(kb == qt))
                rinv = p_ln.tile([P, 1], F32, tag="rinv", name=f"ri_{mt}_{lh}")
                nc.vector.reciprocal(rinv[:], psO[:, P:P + 1])
                # aosb = attn_true (v carries the 16x weight scale; /16 here)
                nc.vector.tensor_scalar(aosb[mt][:, lh, :], psO[:, 0:P],
                                        rinv[:], 1.0 / 16.0,
                                        mybir.AluOpType.mult, mybir.AluOpType.mult)
                if lh == HL - 1:
                    # transpose [tok, 2*128f] -> [2, 128f, tok], cast fp8, stage
                    ptp = ps_sm.tile([P, HL * P], BF16, tag="pssm",
                                     name=f"aop_{mt}")
                    for hh in range(HL):
                        nc.tensor.transpose(ptp[:, P * hh:P * (hh + 1)],
                                            aosb[mt][:, hh, :], ident[:])
                    aoT = p_aoT.tile([P, HL, P], BF16, tag="aoT", name=f"aoT_{mt}")
                    nc.scalar.copy(
                        out=aoT[:],
                        in_=ptp[:].rearrange("p (a b) -> p a b", b=P))
                    g, s, jj = mt // 16, mt % 8, (mt // 8) % 2
                    r0 = 512 * s + 256 * jj
                    nc.sync.dma_start(
                        aot_dram[g, r0:r0 + 256, :].rearrange(
                            "(a p) t -> p a t", a=HL),
                        aoT[:])
                    del aosb[mt]

            rg = [list(range(NCORES))]

            def emit_collective(g):
                nc.gpsimd.collective_compute(
                    "AllToAll", mybir.AluOpType.bypass, replica_groups=rg,
                    ins=[aot_dram[g, :, :]], outs=[a2a_dram[g, :, :]])

            # ================= Stage C: O-proj + LN2 per chunk ================
            h2T = p_ali.tile([P, KS, NCHUNK * P], BF16, tag="ali16", name="h2T")
            h2T8 = p_h2T8.tile([P, KS, NCHUNK * P], FP8, tag="h2T8", name="h2T8")
            x2t = [None] * NCHUNK
            c_state = {}

            def emit_xres(j):
                """xres (host: 16*(x+b2), f32) lands directly in the x2 tile."""
                x2t[j] = p_x2.tile([P, H], F32, tag="x2keep", name=f"x2_{j}")
                nc.scalar.dma_start(x2t[j][:], xres_d[P * j:P * (j + 1), :])

            def emit_C1(j):
                """a2a readback (already fp8 lhsT) + O-proj + scaled residual."""
                g, jj = j // 2, j % 2
                afT = p_afT.tile([P, KS, P], FP8, tag="afT", name=f"afT_{j}")
                a2av = a2a_dram[g].rearrange("(s j f p) t -> p s j f t",
                                             s=NCORES, j=2, p=P)
                for fs in range(2):
                    nc.gpsimd.dma_start(
                        out=afT[:].rearrange("p (s f) t -> p s f t",
                                             s=NCORES)[:, :, fs],
                        in_=a2av[:, :, jj, fs])
                x2 = x2t[j]
                for nn in range(4):
                    psn = ps_c.tile([P, 512], F32, tag="psc", name=f"op_{j}_{nn}")
                    for k2 in range(KS // 2):
                        nc.tensor.matmul(
                            psn[:], lhsT=afT[:, 2 * k2:2 * k2 + 2, :],
                            rhs=wo_sb[:, 2 * k2:2 * k2 + 2, 512 * nn:512 * (nn + 1)],
                            perf_mode=mybir.MatmulPerfMode.DoubleRow,
                            start=(k2 == 0), stop=(k2 == KS // 2 - 1))
                    c0 = 512 * nn
                    # x2 = psO2 + 16*(x+b2)  -> 16 * x2_true   (in place)
                    nc.vector.tensor_tensor(
                        x2[:, c0:c0 + 512], psn[:], x2[:, c0:c0 + 512],
                        mybir.AluOpType.add)
                nmu, rstd = layer_norm_stats([(x2, 4)], f"ln2_{j}", tagp="c2",
                                             eps=epsb2)
                c_state[j] = (x2, nmu, rstd)

            def emit_C2(j):
                """LN2 apply + h2 transpose into h2T (bf16) and h2T8 (fp8)."""
                x2, nmu, rstd = c_state.pop(j)
                h2 = p_h.tile([P, H], BF16, tag="ht", name=f"h2_{j}", bufs=2)
                for hh in range(2):
                    ln_apply(h2[:, (H // 2) * hh:(H // 2) * (hh + 1)],
                             x2[:, (H // 2) * hh:(H // 2) * (hh + 1)],
                             nmu, rstd, nc.gpsimd)
                for fg in range(KS // 8):
                    ptp = ps_c.tile([P, 1024], BF16, tag="psc", name=f"h2t_{j}_{fg}")
                    for f4 in range(8):
                        f = 8 * fg + f4
                        nc.tensor.transpose(ptp[:, P * f4:P * (f4 + 1)],
                                            h2[:, P * f:P * (f + 1)], ident[:])
                    nc.vector.tensor_copy(
                        out=h2T[:, 8 * fg:8 * (fg + 1), P * j:P * (j + 1)],
                        in_=ptp[:].rearrange("p (a b) -> p a b", b=P))
                    nc.scalar.copy(
                        out=h2T8[:, 8 * fg:8 * (fg + 1), P * j:P * (j + 1)],
                        in_=ptp[:].rearrange("p (a b) -> p a b", b=P))

            # ================= Stage D: MLP1 (fused, u stays in SBUF) =========
            silu_fn = (mybir.ActivationFunctionType.Sigmoid if sim
                       else mybir.ActivationFunctionType.Silu)
            # u8p: fp8 mid-tiles 0..N2F-1 as DoubleRow pairs; ubf: tiles N2F..63
            u8p = p_u8.tile([P, N2P, 2, 512], FP8, tag="u8p", name="u8p")
            ubf = None   # allocated after attention frees kq_all

            def u_dst(mm, c0, cn):
                if mm < N2F:
                    return u8p[:, mm // 2, mm % 2, c0:c0 + cn]
                return ubf[:, mm - N2F, c0:c0 + cn]

            def emit_D_tile(mm, c0, cn, w1t=None):
                """MLP1 mid-tile mm over token cols [c0, c0+cn)."""
                fp8 = mm < N1F
                if w1t is None:
                    pool, tg = ((p_w1, "w1t") if mm % 2 == 0 else (p_hT, "hT"))
                    q = nc.gpsimd if mm % 2 == 0 else nc.sync
                    if fp8:
                        w1t = pool.tile([P, KS, P], FP8, tag=tg,
                                        name=f"w1t_{mm}_{c0}")
                        q.dma_start(out=w1t[:], in_=w1f8_d[mm, :, :, :])
                    else:
                        w1t = pool.tile([P, KS, P], BF16, tag=tg,
                                        name=f"w1t_{mm}_{c0}")
                        q.dma_start(out=w1t[:], in_=w1bf_d[mm - N1F, :, :, :])
                if cn == 512:
                    ps = ps_qs.tile([P, 512], F32, tag="psqs", name=f"u_{mm}")
                else:
                    ps = ps_qs.tile([P, cn], F32, tag="psqs", name=f"u_{mm}_{c0}")
                if fp8:
                    for k2 in range(KS // 2):
                        nc.tensor.matmul(
                            ps[:], lhsT=w1t[:, 2 * k2:2 * k2 + 2, :],
                            rhs=h2T8[:, 2 * k2:2 * k2 + 2, c0:c0 + cn],
                            perf_mode=mybir.MatmulPerfMode.DoubleRow,
                            start=(k2 == 0), stop=(k2 == KS // 2 - 1))
                    sc = 1.0 / 16.0
                else:
                    for ks in range(KS):
                        nc.tensor.matmul(ps[:], lhsT=w1t[:, ks, :],
                                         rhs=h2T[:, ks, c0:c0 + cn],
                                         start=(ks == 0), stop=(ks == KS - 1))
                    sc = 1.0
                nc.scalar.activation(u_dst(mm, c0, cn), ps[:], silu_fn,
                                     bias=b1_sb[:, mm:mm + 1], scale=sc)
                return w1t

            # ================= Stage E: MLP2 (4 feature passes) ===============
            def emit_E_pass(p):
                psY = [ps_qs.tile([P, 512], F32, tag="psqs", name=f"y_{p}_{jj}")
                       for jj in range(4)]
                nunit = N2P + NBF2
                for un in range(nunit):
                    pool, tg = ((p_w2, "w2t") if un % 2 == 0 else (p_vsb, "vsb"))
                    q = nc.sync if un % 2 == 0 else nc.scalar
                    if un < N2P:
                        w2t = pool.tile([P, 2, 512], FP8, tag=tg,
                                        name=f"w2t_{p}_{un}")
                        q.dma_start(out=w2t[:], in_=w2f8_d[p, un, :, :, :])
                        for jj in range(4):
                            nc.tensor.matmul(
                                psY[jj][:],
                                lhsT=u8p[:, un, :, P * jj:P * (jj + 1)],
                                rhs=w2t[:],
                                perf_mode=mybir.MatmulPerfMode.DoubleRow,
                                start=(un == 0), stop=(un == nunit - 1))
                    else:
                        w2t = pool.tile([P, 512], BF16, tag=tg,
                                        name=f"w2t_{p}_{un}")
                        q.dma_start(out=w2t[:],
                                    in_=w2bf_d[p, un - N2P, :, :])
                        for jj in range(4):
                            nc.tensor.matmul(
                                psY[jj][:],
                                lhsT=ubf[:, un - N2P, P * jj:P * (jj + 1)],
                                rhs=w2t[:],
                                start=(un == 0), stop=(un == nunit - 1))
                for jj in range(4):
                    # out = (psY + 16*x2_true) / 16: add in psum, scaled copy
                    nc.vector.tensor_tensor(
                        psY[jj][:], psY[jj][:], x2t[jj][:, 512 * p:512 * (p + 1)],
                        mybir.AluOpType.add)
                    ot = p_x.tile([P, 512], BF16, tag="xt", name=f"ot_{p}_{jj}", bufs=6)
                    nc.scalar.activation(ot[:], psY[jj][:],
                                         mybir.ActivationFunctionType.Copy,
                                         scale=1.0 / X2S)
                    nc.sync.dma_start(
                        out=out_d[P * jj:P * (jj + 1), 512 * p:512 * (p + 1)],
                        in_=ot[:])

            # ================= emission schedule ==============================
            for t in range(2):
                xpre[t] = emit_x_tile(t)
            emit_weight_dmas()
            for g in range(NG_PER_B):
                for tt in range(2):
                    tn = 2 * (g + 1) + tt
                    if tn < 16:
                        xpre[tn] = emit_x_tile(tn)
                emit_A_group(0, g)

            # attention(b0) interleaved with QKV(b1)
            for qt in range(QT_PER_B):
                if qt % 2 == 0:
                    for tt in range(2):
                        xpre[16 + qt + tt] = emit_x_tile(16 + qt + tt)
                else:
                    emit_A_group(1, qt // 2)
                exs = [emit_B_S(0, qt, lh) for lh in range(HL)]
                for lh in range(HL):
                    emit_B_AV(0, qt, lh, exs[lh])
                if qt == 1:
                    nc.scalar.dma_start(out=wo_sb[:], in_=wo_d[:, :, :])
                if qt == 3:
                    for j in range(NCHUNK):
                        emit_xres(j)

            # attention(b1); the b0 AllToAll fires after 8 b1 q-tiles so the
            # PE has substantial SBUF-resident work during the mesh entry
            w1pre = {}

            def prefetch_w1(mm):
                pool, tg = ((p_w1, "w1t") if mm % 2 == 0 else (p_hT, "hT"))
                q = nc.gpsimd if mm % 2 == 0 else nc.sync
                w1t = pool.tile([P, KS, P], FP8, tag=tg, name=f"w1p_{mm}")
                q.dma_start(out=w1t[:], in_=w1f8_d[mm, :, :, :])
                w1pre[mm] = w1t

            for qt in range(QT_PER_B):
                exs = [emit_B_S(1, qt, lh) for lh in range(HL)]
                for lh in range(HL):
                    emit_B_AV(1, qt, lh, exs[lh])
                if qt == 7:
                    emit_collective(0)
                if qt == 12:
                    emit_C1(0)
                if qt == 13:
                    emit_C2(0)
                    for mm in range(4):
                        prefetch_w1(mm)
                if qt == 14:
                    emit_C1(1)
                    for mm in range(4, 8):
                        prefetch_w1(mm)
                if qt == 15:
                    emit_C2(1)
            emit_collective(1)

            # early MLP1: fp8 tiles, first token half (chunks 0,1) -- fills the
            # AllToAll latency window; their W1 tiles are streamed again for
            # the second half (cheap: 5MB fp8)
            for mm in range(N1F):
                emit_D_tile(mm, 0, 256, w1t=w1pre.pop(mm, None))
            emit_C1(2)
            emit_C2(2)
            emit_C1(3)
            emit_C2(3)
            # now the kq region is dead (attention complete) -> bf16 u tiles
            ubf = p_kvu.tile([P, MMT - N2F, 512], BF16, tag="kvu", name="ubf")
            for mm in range(N1F):
                emit_D_tile(mm, 256, 256)
            for mm in range(N1F, MMT):
                emit_D_tile(mm, 0, 512)
            for p in range(NPASS):
                emit_E_pass(p)
    nc.compile()
    return nc


def _bf16(a):
    return np.asarray(a, dtype=np.float32).astype(ml_dtypes.bfloat16)


def _fp8(a):
    return np.clip(np.asarray(a, np.float32), -240, 240).astype(mybir.dt.np(FP8))


def make_in_maps(x, Wq, Wk, Wv, Wo, g1, bn1, g2, bn2, W1, b1, W2, b2):
    x = np.asarray(x, np.float32)
    x_flat = np.ascontiguousarray(x.reshape(NTOK, H))

    wq_eff = (g1[:, None] * np.asarray(Wq, np.float32)) * WSCALE
    wk_eff = (g1[:, None] * np.asarray(Wk, np.float32)) * WSCALE
    wv_eff = (g1[:, None] * np.asarray(Wv, np.float32)) * WSCALE
    bq = (bn1 @ np.asarray(Wq, np.float32)) * WSCALE
    bk = (bn1 @ np.asarray(Wk, np.float32)) * WSCALE
    bv = (bn1 @ np.asarray(Wv, np.float32)) * WSCALE
    w1_eff = g2[:, None] * np.asarray(W1, np.float32)
    b1_eff = np.asarray(b1, np.float32) + bn2 @ np.asarray(W1, np.float32)

    xbf = np.ascontiguousarray(_fp8(x_flat))
    # W1: [mm, p, ks, mw]; tiles 0..N1F-1 fp8 (x16), rest bf16
    w1_t = _bf16(w1_eff).astype(np.float32).reshape(KS, P, MMT, P).transpose(2, 1, 0, 3)
    w1f8 = np.ascontiguousarray(_fp8(16.0 * w1_eff.reshape(KS, P, MMT, P)
                                     .transpose(2, 1, 0, 3)[:N1F]))
    w1bf = np.ascontiguousarray(_bf16(w1_eff.reshape(KS, P, MMT, P)
                                      .transpose(2, 1, 0, 3)[N1F:]))
    # W2 scaled by 32 on both dtypes; [pass][unit][...]
    W2f = np.asarray(W2, np.float32) * 16.0
    w2f8 = np.empty((NPASS, N2P, P, 2, 512), mybir.dt.np(FP8))
    w2bf = np.empty((NPASS, NBF2, P, 512), ml_dtypes.bfloat16)
    for p in range(NPASS):
        cols = slice(512 * p, 512 * (p + 1))
        for q in range(N2P):
            w2f8[p, q, :, 0, :] = _fp8(W2f[P * 2 * q:P * (2 * q + 1), cols])
            w2f8[p, q, :, 1, :] = _fp8(W2f[P * (2 * q + 1):P * (2 * q + 2), cols])
        for i in range(NBF2):
            mm = N2F + i
            w2bf[p, i] = _bf16(W2f[P * mm:P * (mm + 1), cols])
    b1m = np.ascontiguousarray(b1_eff.reshape(MMT, P).T.astype(np.float32))
    wo8 = np.ascontiguousarray(
        _fp8(16.0 * np.asarray(Wo, np.float32)).reshape(KS, P, H).transpose(1, 0, 2))
    ii, jj_ = np.meshgrid(np.arange(P), np.arange(P), indexing="ij")
    cmaskT = np.where(ii <= jj_, 0.0, NEG).astype(np.float32)
    b2f = np.asarray(b2, np.float32)

    in_maps = []
    for c in range(NCORES):
        cs = slice(DV * c, DV * (c + 1))
        wqk = np.concatenate([wq_eff[:, cs], wk_eff[:, cs]], axis=1)  # [H, 512]
        wqk_t = np.ascontiguousarray(
            _fp8(wqk).reshape(KS, P, DQK).transpose(1, 0, 2))
        bqk = np.concatenate([bq[cs], bk[cs]]).astype(np.float32)
        bqk_m = np.ascontiguousarray(bqk.reshape(DQK // P, P).T)
        wv_t = np.ascontiguousarray(
            _fp8(wv_eff[:, cs]).reshape(KS, P, DV).transpose(1, 0, 2))
        bvbc = np.ascontiguousarray(np.broadcast_to(
            bv[cs].astype(np.float32).reshape(1, HL, P), (P, HL, P)))
        xres = np.concatenate(
            [x_flat[1024 * j + P * c:1024 * j + P * (c + 1)] for j in range(NCHUNK)],
            axis=0) + b2f
        xres16 = np.ascontiguousarray((16.0 * xres).astype(np.float32))
        in_maps.append({
            "xbf": xbf, "xres": xres16,
            "wqk": wqk_t, "bqk": bqk_m, "wv": wv_t, "bvbc": bvbc, "wo": wo8,
            "w1f8": w1f8, "w1bf": w1bf, "b1": b1m, "w2f8": w2f8, "w2bf": w2bf,
            "cmaskT": cmaskT,
        })
    return in_maps


_NC_CACHE = {}


def kernel(**inputs):
    if "nc" not in _NC_CACHE:
        _NC_CACHE["nc"] = build()
    nc = _NC_CACHE["nc"]
    in_maps = make_in_maps(
        inputs["x"], inputs["Wq"], inputs["Wk"], inputs["Wv"], inputs["Wo"],
        np.asarray(inputs["g1"], np.float32), np.asarray(inputs["bn1"], np.float32),
        np.asarray(inputs["g2"], np.float32), np.asarray(inputs["bn2"], np.float32),
        inputs["W1"], inputs["b1"], inputs["W2"], inputs["b2"])
    res = run_bass_kernel_spmd(nc, in_maps, list(range(NCORES)))
    out = np.empty((NTOK, H), np.float32)
    for c in range(NCORES):
        oc = np.asarray(res.results[c]["out"], dtype=np.float32)
        for j in range(NCHUNK):
            out[1024 * j + P * c:1024 * j + P * (c + 1)] = oc[P * j:P * (j + 1)]
    return out.reshape(B, T, H)
